# revision 22
# baseline (speedup 1.0000x reference)
"""GCN forward (4-layer GCNConv + global mean-pool + linear) on 8 TRN2 cores.

Redesign vs. the v1 kernel (dst-tile dma_gather at 256B/edge, 4 layers):
  * Layer 4 + mean-pool are collapsed into a host-built structure matrix
    P[graph, node] (= pool(1/cnt) . A_hat norms): pooled = (P @ h3) @ W4 + b4.
    The widest aggregation (F=64) becomes a tiny dense TensorE contraction.
  * Gather rows are packed: h stored contiguously [Npad, F] bf16 so one 256B
    dma_gather row holds k = 128/F nodes (16/8/4 for F=8/16/32). An int16 row
    index then covers all of Npad -> no src chunking, and per-(tile) padding
    only (to 128) -> ~213k descriptors/layer/core vs 416k before.
  * Per-edge sub-row selection is a DVE mask (sub == iota_k) multiplied into
    the gathered rows; the one-hot scatter matmul then accumulates a full
    [128 dst, 128col] psum whose k F-wide column blocks are slice-reduced.
  * Self-loop term is computed directly as dinv^2 * h per dst tile (DVE),
    never gathered.
  * Aggregation math: agg_d = dinv_d * sum_{s->d} (dinv_s h_s) + dinv_d^2 h_d,
    with dinv_s folded into the stored gather source.

All graph preprocessing (tile packing, index/mask tables, P) is host numpy
and depends only on graph structure (edges/batch/degrees), never on x or W.
"""

import heapq

import numpy as np
import ml_dtypes

import concourse.bacc as bacc
import concourse.mybir as mybir
import concourse.tile as tile
from concourse.bass_utils import run_bass_kernel_spmd
from concourse.library_config import mlp as mlp_lib
from concourse.masks import make_identity

F32 = mybir.dt.float32
BF16 = mybir.dt.bfloat16
I16 = mybir.dt.int16

NCORES = 8
TT = 128              # dst tiles per core
NPC = TT * 128        # dst nodes per core (16384)
GIDX = 1024           # max idxs per dma_gather instruction (SWDGE ring limit)
PAD_S = 255.0         # slot/sub id for padding positions


# ------------------------------------------------------------------ host prep
def _preprocess(x, edge_src, edge_dst, batch, num_graphs):
    N = x.shape[0]
    Npad = NCORES * NPC
    indeg = np.bincount(edge_dst, minlength=N).astype(np.int64)
    deg = indeg + 1  # self loop
    dinv = (1.0 / np.sqrt(deg.astype(np.float64))).astype(np.float32)

    # Greedy balanced assignment of dst nodes to (core, tile) bins by indegree.
    order = np.argsort(-indeg, kind="stable")
    nbins = NCORES * TT
    heap = [(0, b) for b in range(nbins)]
    heapq.heapify(heap)
    counts = np.zeros(nbins, np.int64)
    gid = np.empty(N, np.int64)
    for v in order:
        load, b = heapq.heappop(heap)
        c, t = b // TT, b % TT
        gid[v] = c * NPC + counts[b] * TT + t
        counts[b] += 1
        if counts[b] < 128:
            heapq.heappush(heap, (load + int(indeg[v]), b))
    assert counts.max() <= 128

    dinv_pad = np.ones(Npad, np.float32)
    dinv_pad[gid] = dinv

    sg, dg = gid[edge_src], gid[edge_dst]
    core_e = dg // NPC
    tile_e = dg % TT
    slot_e = (dg % NPC) // TT
    binid = core_e * TT + tile_e
    cnts = np.bincount(binid, minlength=nbins)
    L = int(-(-cnts.max() // 128) * 128)
    S = TT * L

    ks = (16, 8, 4)
    order_e = np.argsort(binid, kind="stable")
    sortedbin = binid[order_e]
    run_start = np.searchsorted(sortedbin, np.arange(nbins))
    rank = np.arange(len(order_e)) - run_start[sortedbin]
    pos = (sortedbin % TT) * L + rank
    core_arr = sortedbin // TT

    idx_flat = np.zeros((3, NCORES, S), np.int16)
    sub_flat = np.full((3, NCORES, S), PAD_S, np.float32)
    sval_flat = np.full((NCORES, S), PAD_S, np.float32)
    for c in range(NCORES):
        m = core_arr == c
        es, p = order_e[m], pos[m]
        sval_flat[c, p] = slot_e[es].astype(np.float32)
        for li, k in enumerate(ks):
            idx_flat[li, c, p] = (sg[es] // k).astype(np.int16)
            sub_flat[li, c, p] = (sg[es] % k).astype(np.float32)

    def wrap16(a):  # [S] -> [128, S//16]
        return np.tile(a.reshape(S // 16, 16).T, (8, 1)).copy()

    def wrap128(a):  # [S] -> [128, S//128]
        return a.reshape(S // 128, 128).T.astype(ml_dtypes.bfloat16)

    idx_tbl = np.stack([[wrap16(idx_flat[li, c]) for c in range(NCORES)]
                        for li in range(3)])
    sub_tbl = np.stack([[wrap128(sub_flat[li, c]) for c in range(NCORES)]
                        for li in range(3)])
    sval_tbl = np.stack([wrap128(sval_flat[c]) for c in range(NCORES)])

    x_perm = np.zeros((Npad, x.shape[1]), np.float32)
    x_perm[gid] = x
    dinv_all = dinv_pad.reshape(128, Npad // 128).astype(ml_dtypes.bfloat16)
    dinv_my = dinv_pad.reshape(NCORES, 128, TT).copy()
    dinv2_my = (dinv_my * dinv_my).copy()
    x_my = x_perm.reshape(NCORES, 128, TT * x.shape[1]).copy()    # [8,128,TT*8]

    cnt = np.bincount(batch, minlength=num_graphs).astype(np.float64)
    invc = (1.0 / np.maximum(cnt, 1.0)).astype(np.float64)
    bd = batch[edge_dst].astype(np.int64)
    w = dinv[edge_src].astype(np.float64) * dinv[edge_dst] * invc[bd]
    Pacc = np.bincount(sg * num_graphs + bd, weights=w,
                       minlength=Npad * num_graphs)
    Pacc += np.bincount(gid * num_graphs + batch.astype(np.int64),
                        weights=(dinv.astype(np.float64) ** 2) * invc[batch],
                        minlength=Npad * num_graphs)
    Pacc = Pacc.reshape(Npad, num_graphs).astype(np.float32)
    P_my = Pacc.reshape(NCORES, 128, TT * num_graphs).astype(ml_dtypes.bfloat16)

    x_perm = x_perm.astype(ml_dtypes.bfloat16)
    return dict(L=L, S=S, Npad=Npad, idx_tbl=idx_tbl, sub_tbl=sub_tbl,
                sval_tbl=sval_tbl, x_perm=x_perm, x_my=x_my,
                dinv_all=dinv_all, dinv_my=dinv_my, dinv2_my=dinv2_my,
                P_my=P_my)


# ------------------------------------------------------------------ device IR
def _build(meta, num_graphs, n_classes, alphas, n_cores=NCORES):
    L, S, Npad = meta["L"], meta["S"], meta["Npad"]
    Fs = [8, 16, 32]          # aggregation widths, layers 1-3
    Fos = [16, 32, 64]        # output widths, layers 1-3
    ks = [16, 8, 4]           # nodes per 256B gather row
    nodes_my = NPC

    nc = bacc.Bacc("TRN2", target_bir_lowering=False, debug=False,
                   num_devices=n_cores, num_swdge_queues=4)
    rg = [list(range(n_cores))]

    x_in = nc.dram_tensor("x_perm", [Npad, 8], BF16, kind="ExternalInput")
    xmy_in = nc.dram_tensor("x_my", [128, TT * 8], F32, kind="ExternalInput")
    dall_in = nc.dram_tensor("dinv_all", [128, Npad // 128], BF16,
                             kind="ExternalInput")
    dmy_in = nc.dram_tensor("dinv_my", [128, TT], F32, kind="ExternalInput")
    d2my_in = nc.dram_tensor("dinv2_my", [128, TT], F32, kind="ExternalInput")
    idx_in = [nc.dram_tensor(f"idx{l+1}", [128, S // 16], I16,
                             kind="ExternalInput") for l in range(3)]
    sub_in = [nc.dram_tensor(f"sub{l+1}", [128, S // 128], BF16,
                             kind="ExternalInput") for l in range(3)]
    sval_in = nc.dram_tensor("sval", [128, S // 128], BF16, kind="ExternalInput")
    P_in = nc.dram_tensor("P_my", [128, TT * num_graphs], BF16,
                          kind="ExternalInput")
    W_in = [nc.dram_tensor(f"W{l+1}", [Fs[l], Fos[l]], BF16,
                           kind="ExternalInput") for l in range(3)]
    b_in = [nc.dram_tensor(f"b{l+1}", [Fos[l], 1], F32, kind="ExternalInput")
            for l in range(3)]
    bn_in = [nc.dram_tensor(f"bn{l+1}", [Fos[l], 1], F32, kind="ExternalInput")
             for l in range(3)]
    W4_in = nc.dram_tensor("W4", [64, 128], F32, kind="ExternalInput")
    b4_in = nc.dram_tensor("b4", [128, 1], F32, kind="ExternalInput")
    Wlin_in = nc.dram_tensor("Wlin", [128, n_classes], F32, kind="ExternalInput")
    blin_in = nc.dram_tensor("blin_rep", [num_graphs, n_classes], F32,
                             kind="ExternalInput")
    out_t = nc.dram_tensor("out", [num_graphs, n_classes], F32,
                           kind="ExternalOutput")

    g1 = nc.dram_tensor("g1", [Npad, 8], BF16)
    g2 = nc.dram_tensor("g2", [Npad, 16], BF16)
    g3 = nc.dram_tensor("g3", [Npad, 32], BF16)
    gs = [g1, g2, g3]
    hs2 = nc.dram_tensor("hs2", [nodes_my, 16], BF16)
    hs3 = nc.dram_tensor("hs3", [nodes_my, 32], BF16)
    hss = [hs2, hs3]
    pooled_d = nc.dram_tensor("pooled", [128, num_graphs], F32)
    pooled_r = nc.dram_tensor("pooled_red", [128, num_graphs], F32)

    with tile.TileContext(nc) as tc:
        with (
            tc.tile_pool(name="const", bufs=1) as cpool,
            tc.tile_pool(name="meta", bufs=2) as mpool,
            tc.tile_pool(name="gat", bufs=6) as gpool,
            tc.tile_pool(name="am", bufs=6) as apool,
            tc.tile_pool(name="red", bufs=4) as rpool,
            tc.tile_pool(name="big", bufs=1) as bpool,
            tc.tile_pool(name="ps", bufs=1, space="PSUM") as pspool,
        ):
            nc.gpsimd.load_library(mlp_lib)

            iden = cpool.tile([128, 128], BF16)
            make_identity(nc, iden[:])
            iota = cpool.tile([128, 128], BF16)
            nc.gpsimd.iota(iota[:], [[1, 128]], channel_multiplier=0,
                           allow_small_or_imprecise_dtypes=True)

            dinv_my = cpool.tile([128, TT], F32)
            nc.sync.dma_start(dinv_my[:], dmy_in.ap())
            dinv2_my = cpool.tile([128, TT], F32)
            nc.sync.dma_start(dinv2_my[:], d2my_in.ap())
            sval_sb = cpool.tile([128, S // 128], BF16)
            nc.sync.dma_start(sval_sb[:], sval_in.ap())
            x_my = cpool.tile([128, TT * 8], F32)
            nc.sync.dma_start(x_my[:], xmy_in.ap())
            P_sb = cpool.tile([128, TT * num_graphs], BF16)
            nc.sync.dma_start(P_sb[:], P_in.ap())

            Wt, btl, bntl = [], [], []
            for l in range(3):
                w = cpool.tile([128, Fos[l]], BF16, tag=f"W{l}")
                nc.sync.dma_start(w[:Fs[l], :], W_in[l].ap())
                Wt.append(w)
                b = cpool.tile([128, 1], F32, tag=f"b{l}")
                nc.sync.dma_start(b[:Fos[l], :], b_in[l].ap())
                btl.append(b)
                bn = cpool.tile([128, 1], F32, tag=f"bn{l}")
                nc.sync.dma_start(bn[:Fos[l], :], bn_in[l].ap())
                bntl.append(bn)
            W4_sb = cpool.tile([128, 128], F32, tag="W4")
            nc.sync.dma_start(W4_sb[:64, :], W4_in.ap())
            b4_sb = cpool.tile([128, 1], F32, tag="b4")
            nc.sync.dma_start(b4_sb[:], b4_in.ap())
            Wlin_sb = cpool.tile([128, n_classes], F32, tag="wlin")
            nc.sync.dma_start(Wlin_sb[:], Wlin_in.ap())
            blin_sb = cpool.tile([num_graphs, n_classes], F32, tag="blin")
            nc.sync.dma_start(blin_sb[:], blin_in.ap())

            # ---------------- g1 = dinv * x (full, every core), bf16 packed
            ncols = Npad // 128
            CCH = 128
            xv = x_in.ap().rearrange("(p c) f -> p c f", p=128)
            g1v = g1.ap().rearrange("(p c) f -> p c f", p=128)
            for c0 in range(0, ncols, CCH):
                xt = mpool.tile([128, CCH, 8], BF16, tag="xt")
                nc.sync.dma_start(xt[:], xv[:, c0:c0 + CCH, :])
                da = mpool.tile([128, CCH], BF16, tag="da")
                nc.sync.dma_start(da[:], dall_in.ap()[:, c0:c0 + CCH])
                gt = mpool.tile([128, CCH, 8], BF16, tag="gt")
                nc.vector.tensor_tensor(
                    gt[:], xt[:],
                    da[:, :, None].broadcast_to([128, CCH, 8]),
                    op=mybir.AluOpType.mult)
                nc.sync.dma_start(g1v[:, c0:c0 + CCH, :], gt[:])

            gq = [0]
            aggT = bpool.tile([128, nodes_my // 2], BF16, tag="aggT")
            h_sbT = bpool.tile([128, nodes_my // 2], BF16, tag="h_sbT")
            agg = bpool.tile([128, TT * 32], BF16, tag="agg")
            gnext = bpool.tile([128, TT * 32], BF16, tag="gnext")
            h_resA = bpool.tile([128, TT * 64], BF16, tag="h_resA")
            h_resB = bpool.tile([128, TT * 32], BF16, tag="h_resB")
            idx_h = [bpool.tile([128, S // 32], I16, tag=f"idx_h{h}",
                                name=f"idx_h{h}") for h in range(2)]
            sub_d = [bpool.tile([128, S // 128], BF16, tag=f"sub_d{h}",
                                name=f"sub_d{h}") for h in range(2)]
            nc.sync.dma_start(idx_h[0][:], idx_in[0].ap()[:, :S // 32])
            nc.sync.dma_start(idx_h[1][:], idx_in[0].ap()[:, S // 32:])
            nc.sync.dma_start(sub_d[0][:], sub_in[0].ap())
            pp_sb = cpool.tile([128, num_graphs], F32, tag="pp_sb")
            nc.vector.memset(pp_sb[:], 0.0)

            for li in range(3):
                F, Fo, k = Fs[li], Fos[li], ks[li]
                h_res_prev = [None, h_resA, h_resB][li]
                h_res_next = [h_resA, h_resB, h_resA][li]
                sub_sb = sub_d[li % 2]
                gsrc = gs[li].ap().rearrange("(r k) f -> r (k f)", k=k)

                # ---- aggregation + post-phase, interleaved per node-half.
                # Flat 1024-idx gather stream; groups map to dst tiles via
                # position (tile t spans [t*L, (t+1)*L), L % 128 == 0).
                a_f = alphas[li]
                HT = TT // 2
                SH = S // 2
                psums = {}

                def finish_tile(t, k=k, F=F, li=li,
                                h_res_prev=h_res_prev):
                    # drain psum -> sbuf, slice-reduce k blocks, self-loop
                    psum = psums.pop(t)
                    red0 = rpool.tile([128, 128], F32, tag="red0")
                    nc.scalar.copy(red0[:], psum[:, :128])
                    cur = red0[:].rearrange("p (k f) -> p k f", k=k)
                    kk = k
                    while kk > 1:
                        half = kk // 2
                        nxt = rpool.tile([128, 8, 32], F32, tag=f"red{kk}")
                        nc.vector.tensor_tensor(
                            nxt[:, :half, :F], cur[:, :half, :],
                            cur[:, half:kk, :], op=mybir.AluOpType.add)
                        cur = nxt[:, :half, :F]
                        kk = half
                    sc = rpool.tile([128, 32], F32, tag="sc")
                    hp_src = x_my if li == 0 else h_res_prev
                    nc.vector.tensor_tensor(
                        sc[:, :F], hp_src[:, t * F:(t + 1) * F],
                        dinv2_my[:, t:t + 1].broadcast_to([128, F]),
                        op=mybir.AluOpType.mult)
                    t2 = rpool.tile([128, 32], F32, tag="t2")
                    nc.scalar.activation(
                        t2[:, :F], cur[:, 0, :],
                        mybir.ActivationFunctionType.Identity,
                        scale=dinv_my[:, t:t + 1])
                    nc.vector.tensor_tensor(
                        agg[:, t * F:(t + 1) * F], t2[:, :F], sc[:, :F],
                        op=mybir.AluOpType.add)

                for hb in range(2):
                  if True:
                    idx_sb = idx_h[hb]
                    for base in range(hb * SH, (hb + 1) * SH, GIDX):
                        nidx = GIDX
                        ng = nidx // 128
                        gt = gpool.tile([128, 8, 128], BF16, tag="gtile")
                        nc.gpsimd.dma_gather(
                            gt[:, :ng, :], gsrc,
                            idx_sb[:, (base - hb * SH) // 16:
                                   (base - hb * SH + nidx) // 16],
                            nidx, nidx, 128, queue_num=gq[0] % 4)
                        gq[0] += 1
                        sv = sval_sb[:, base // 128:(base + nidx) // 128]
                        sb = sub_sb[:, base // 128:(base + nidx) // 128]
                        A = apool.tile([128, 8, 128], BF16, tag="A")
                        nc.vector.tensor_tensor(
                            A[:, :ng, :],
                            sv[:, :, None].broadcast_to([128, ng, 128]),
                            iota[:, None, :].broadcast_to([128, ng, 128]),
                            op=mybir.AluOpType.is_equal)
                        Mt = apool.tile([128, 128], BF16, tag="M")
                        nc.vector.tensor_tensor(
                            Mt[:, :ng * k].rearrange("p (g k) -> p g k", k=k),
                            sb[:, :, None].broadcast_to([128, ng, k]),
                            iota[:, None, :k].broadcast_to([128, ng, k]),
                            op=mybir.AluOpType.is_equal)
                        Gm = gpool.tile([128, 8, 128], BF16, tag="Gm")
                        nc.vector.tensor_tensor(
                            Gm[:, :ng, :].rearrange("p g (k f) -> p (g k) f",
                                                    k=k),
                            gt[:, :ng, :].rearrange("p g (k f) -> p (g k) f",
                                                    k=k),
                            Mt[:, :ng * k, None].broadcast_to([128, ng * k, F]),
                            op=mybir.AluOpType.mult)
                        for gg in range(ng):
                            pos = base + gg * 128
                            t = pos // L
                            if t not in psums:
                                psums[t] = pspool.tile(
                                    [128, 512], F32, tag=f"ps{t % 6}",
                                    name=f"pst{t % 6}")
                            nc.tensor.matmul(
                                psums[t][:, :128], A[:, gg, :], Gm[:, gg, :],
                                start=(pos % L == 0),
                                stop=(pos % L == L - 128),
                                skip_group_check=True)
                            if pos % L == L - 128:
                                finish_tile(t)
                    if li < 2:
                        nc.sync.dma_start(
                            idx_h[hb][:],
                            idx_in[li + 1].ap()[:, hb * (S // 32):
                                                (hb + 1) * (S // 32)])
                        if hb == 0:
                            nc.sync.dma_start(sub_d[(li + 1) % 2][:],
                                              sub_in[li + 1].ap())

                  # ---- this half: transpose agg -> aggT, W matmul+PReLU,
                  #      transpose back, keep h_res, emit gnext / P-psum
                  if True:
                    t0 = hb * HT
                    for tt in range(HT):
                        t = t0 + tt
                        tp = pspool.tile([128, 512], BF16, tag=f"ps{6 + tt % 2}")
                        nc.tensor.matmul(tp[:F, :128],
                                         agg[:, t * F:(t + 1) * F],
                                         iden[:], is_transpose=True,
                                         skip_group_check=True)
                        nc.scalar.copy(aggT[:F, tt * 128:(tt + 1) * 128],
                                       tp[:F, :128])
                    for n0 in range(0, nodes_my // 2, 512):
                        hp = pspool.tile([128, 512], F32,
                                         tag=f"ps{6 + (n0 // 512) % 2}")
                        nc.tensor.matmul(hp[:Fo, :512], Wt[li][:F, :Fo],
                                         aggT[:F, n0:n0 + 512],
                                         skip_group_check=True)
                        # prelu(x+b) = relu(x+b) - a * relu(-x-b)
                        nc.scalar.activation(
                            h_sbT[:Fo, n0:n0 + 512], hp[:Fo, :512],
                            mybir.ActivationFunctionType.Relu,
                            bias=btl[li][:Fo, :], scale=1.0)
                        hrelu = mpool.tile([128, 512], BF16, tag="hrelu")
                        nc.scalar.activation(
                            hrelu[:Fo, :512], hp[:Fo, :512],
                            mybir.ActivationFunctionType.Relu,
                            bias=bntl[li][:Fo, :], scale=-1.0)
                        nc.vector.scalar_tensor_tensor(
                            h_sbT[:Fo, n0:n0 + 512], hrelu[:Fo, :512],
                            float(-a_f), h_sbT[:Fo, n0:n0 + 512],
                            op0=mybir.AluOpType.mult, op1=mybir.AluOpType.add)
                    for tt in range(HT):
                        t = t0 + tt
                        tb = pspool.tile([128, 512], BF16, tag=f"ps{6 + tt % 2}")
                        nc.tensor.matmul(tb[:128, :Fo],
                                         h_sbT[:Fo, tt * 128:(tt + 1) * 128],
                                         iden[:Fo, :Fo], is_transpose=True,
                                         skip_group_check=True)
                        nc.vector.tensor_copy(
                            h_res_next[:, t * Fo:(t + 1) * Fo], tb[:, :Fo])
                        if li < 2:
                            nc.scalar.activation(
                                gnext[:, t * Fo:(t + 1) * Fo], tb[:, :Fo],
                                mybir.ActivationFunctionType.Identity,
                                scale=dinv_my[:, t:t + 1])
                        else:
                            ppp = pspool.tile([128, 512], F32,
                                              tag=f"ps{6 + (tt + 1) % 2}")
                            nc.tensor.matmul(
                                ppp[:64, :num_graphs],
                                h_res_next[:, t * Fo:(t + 1) * Fo],
                                P_sb[:, t * num_graphs:(t + 1) * num_graphs],
                                start=True, stop=True,
                                skip_group_check=True)
                            nc.vector.tensor_tensor(
                                pp_sb[:64, :], pp_sb[:64, :],
                                ppp[:64, :num_graphs],
                                op=mybir.AluOpType.add)

                    if li < 2:
                        hsv = hss[li].ap().rearrange("(p t) f -> p t f", p=128)
                        nc.sync.dma_start(
                            hsv[:, t0:t0 + HT, :],
                            gnext[:, t0 * Fo:(t0 + HT) * Fo].rearrange(
                                "p (t f) -> p t f", f=Fo))
                if li < 2:
                    if n_cores > 1:
                        nc.gpsimd.collective_compute(
                            "AllGather", mybir.AluOpType.bypass, rg,
                            [hss[li].ap()], [gs[li + 1].ap()])
                    else:
                        nc.sync.dma_start(gs[li + 1].ap()[:nodes_my, :],
                                          hss[li].ap())

            # ---------------- tail: pooledT = W4^T @ ppT + b4; AllReduce; lin
            poolp = pspool.tile([128, 512], F32, tag="ps7")
            nc.tensor.matmul(poolp[:128, :num_graphs], W4_sb[:64, :128],
                             pp_sb[:64, :num_graphs], skip_group_check=True)
            poolT_sb = cpool.tile([128, num_graphs], F32, tag="poolT")
            nc.scalar.activation(poolT_sb[:], poolp[:128, :num_graphs],
                                 mybir.ActivationFunctionType.Identity,
                                 bias=b4_sb[:, :], scale=1.0)
            if n_cores > 1:
                nc.sync.dma_start(pooled_d.ap(), poolT_sb[:])
                nc.gpsimd.collective_compute(
                    "AllReduce", mybir.AluOpType.add, rg,
                    [pooled_d.ap()], [pooled_r.ap()])
                poolT2 = cpool.tile([128, num_graphs], F32, tag="poolT2")
                nc.sync.dma_start(poolT2[:], pooled_r.ap())
            else:
                poolT2 = poolT_sb
            fin = pspool.tile([128, 512], F32, tag="ps6")
            nc.tensor.matmul(fin[:num_graphs, :n_classes], poolT2[:],
                             Wlin_sb[:], skip_group_check=True)
            out_sb = cpool.tile([num_graphs, n_classes], F32, tag="outsb")
            nc.vector.tensor_tensor(out_sb[:], fin[:num_graphs, :n_classes],
                                    blin_sb[:], op=mybir.AluOpType.add)
            nc.sync.dma_start(out_t.ap(), out_sb[:])

    nc.compile()
    return nc


# ------------------------------------------------------------------ entry
def kernel(x, edge_src, edge_dst, batch,
           W1, b1, W2, b2, W3, b3, W4, b4,
           a1, a2, a3, Wlin, blin, n_cores=NCORES):
    x = np.asarray(x, dtype=np.float32)
    edge_src = np.asarray(edge_src, dtype=np.int32)
    edge_dst = np.asarray(edge_dst, dtype=np.int32)
    batch = np.asarray(batch, dtype=np.int32)
    Ws = [np.asarray(w, np.float32) for w in (W1, W2, W3, W4)]
    bs = [np.asarray(b, np.float32) for b in (b1, b2, b3, b4)]
    alphas = [float(a1), float(a2), float(a3)]
    Wlin = np.asarray(Wlin, np.float32)
    blin = np.asarray(blin, np.float32)
    NG, NCLS = 64, Wlin.shape[1]

    meta = _preprocess(x, edge_src, edge_dst, batch, NG)
    nc = _build(meta, NG, NCLS, alphas, n_cores)
    in_maps = _in_maps(meta, Ws, bs, Wlin, blin, NG, n_cores)
    res = run_bass_kernel_spmd(nc, in_maps, core_ids=list(range(n_cores)))
    return np.asarray(res.results[0]["out"], dtype=np.float32)


def _in_maps(meta, Ws, bs, Wlin, blin, NG, n_cores=NCORES):
    in_maps = []
    for c in range(n_cores):
        m = dict(
            x_perm=meta["x_perm"],
            x_my=np.ascontiguousarray(meta["x_my"][c]),
            dinv_all=meta["dinv_all"],
            dinv_my=np.ascontiguousarray(meta["dinv_my"][c]),
            dinv2_my=np.ascontiguousarray(meta["dinv2_my"][c]),
            sval=np.asarray(meta["sval_tbl"][c]),
            P_my=np.asarray(meta["P_my"][c]),
            W4=Ws[3].astype(np.float32),
            b4=np.ascontiguousarray(bs[3].reshape(-1, 1)),
            Wlin=Wlin,
            blin_rep=np.tile(blin[None, :], (NG, 1)).astype(np.float32),
        )
        for l in range(3):
            m[f"idx{l+1}"] = np.asarray(meta["idx_tbl"][l, c])
            m[f"sub{l+1}"] = np.asarray(meta["sub_tbl"][l][c])
            m[f"W{l+1}"] = Ws[l].astype(ml_dtypes.bfloat16)
            m[f"b{l+1}"] = np.ascontiguousarray(bs[l].reshape(-1, 1))
            m[f"bn{l+1}"] = np.ascontiguousarray(-bs[l].reshape(-1, 1))
        in_maps.append(m)
    return in_maps


# revision 23
# speedup vs baseline: 1.0259x; 1.0259x over previous
"""GCN forward (4-layer GCNConv + global mean-pool + linear) on 8 TRN2 cores.

Redesign vs. the v1 kernel (dst-tile dma_gather at 256B/edge, 4 layers):
  * Layer 4 + mean-pool are collapsed into a host-built structure matrix
    P[graph, node] (= pool(1/cnt) . A_hat norms): pooled = (P @ h3) @ W4 + b4.
    The widest aggregation (F=64) becomes a tiny dense TensorE contraction.
  * Gather rows are packed: h stored contiguously [Npad, F] bf16 so one 256B
    dma_gather row holds k = 128/F nodes (16/8/4 for F=8/16/32). An int16 row
    index then covers all of Npad -> no src chunking, and per-(tile) padding
    only (to 128) -> ~213k descriptors/layer/core vs 416k before.
  * Per-edge sub-row selection is a DVE mask (sub == iota_k) multiplied into
    the gathered rows; the one-hot scatter matmul then accumulates a full
    [128 dst, 128col] psum whose k F-wide column blocks are slice-reduced.
  * Self-loop term is computed directly as dinv^2 * h per dst tile (DVE),
    never gathered.
  * Aggregation math: agg_d = dinv_d * sum_{s->d} (dinv_s h_s) + dinv_d^2 h_d,
    with dinv_s folded into the stored gather source.

All graph preprocessing (tile packing, index/mask tables, P) is host numpy
and depends only on graph structure (edges/batch/degrees), never on x or W.
"""

import heapq

import numpy as np
import ml_dtypes

import concourse.bacc as bacc
import concourse.mybir as mybir
import concourse.tile as tile
from concourse.bass_utils import run_bass_kernel_spmd
from concourse.library_config import mlp as mlp_lib
from concourse.masks import make_identity

F32 = mybir.dt.float32
BF16 = mybir.dt.bfloat16
I16 = mybir.dt.int16

NCORES = 8
TT = 128              # dst tiles per core
NPC = TT * 128        # dst nodes per core (16384)
GIDX = 1024           # max idxs per dma_gather instruction (SWDGE ring limit)
PAD_S = 255.0         # slot/sub id for padding positions


# ------------------------------------------------------------------ host prep
def _preprocess(x, edge_src, edge_dst, batch, num_graphs):
    N = x.shape[0]
    Npad = NCORES * NPC
    indeg = np.bincount(edge_dst, minlength=N).astype(np.int64)
    deg = indeg + 1  # self loop
    dinv = (1.0 / np.sqrt(deg.astype(np.float64))).astype(np.float32)

    # Two-tier greedy assignment of dst nodes to (core, tile) bins by
    # indegree: tiles 0..123 target just under 12 gather groups (1536 slots),
    # the last 4 tiles per core absorb the excess -> ~5% fewer pad slots.
    order = np.argsort(-indeg, kind="stable")
    nbins = NCORES * TT
    NHI = 4
    cap_lo = 12 * 128 - 10.0
    t_hi = (len(edge_dst) / NCORES - (TT - NHI) * cap_lo) / NHI
    target = np.full(TT, cap_lo)
    target[TT - NHI:] = max(t_hi, cap_lo)
    heap = [(-target[b % TT], b) for b in range(nbins)]
    heapq.heapify(heap)
    counts = np.zeros(nbins, np.int64)
    loads = np.zeros(nbins, np.float64)
    gid = np.empty(N, np.int64)
    for v in order:
        key, b = heapq.heappop(heap)
        c, t = b // TT, b % TT
        gid[v] = c * NPC + counts[b] * TT + t
        counts[b] += 1
        loads[b] += indeg[v]
        if counts[b] < 128:
            heapq.heappush(heap, (loads[b] - target[t], b))
    assert counts.max() <= 128

    dinv_pad = np.ones(Npad, np.float32)
    dinv_pad[gid] = dinv

    sg, dg = gid[edge_src], gid[edge_dst]
    core_e = dg // NPC
    tile_e = dg % TT
    slot_e = (dg % NPC) // TT
    binid = core_e * TT + tile_e
    cnts = np.bincount(binid, minlength=nbins).reshape(NCORES, TT)
    Ls = (-(-cnts.max(axis=0) // 128) * 128).astype(np.int64)
    Ls[TT // 2 - 1] += (-Ls[:TT // 2].sum()) % 1024     # align half streams
    Ls[TT - 1] += (-Ls[TT // 2:].sum()) % 1024
    prefix = np.concatenate([[0], np.cumsum(Ls)])
    S = int(prefix[-1])

    ks = (16, 8, 4)
    order_e = np.argsort(binid, kind="stable")
    sortedbin = binid[order_e]
    run_start = np.searchsorted(sortedbin, np.arange(nbins))
    rank = np.arange(len(order_e)) - run_start[sortedbin]
    pos = prefix[sortedbin % TT] + rank
    core_arr = sortedbin // TT

    idx_flat = np.zeros((3, NCORES, S), np.int16)
    sub_flat = np.full((3, NCORES, S), PAD_S, np.float32)
    sval_flat = np.full((NCORES, S), PAD_S, np.float32)
    for c in range(NCORES):
        m = core_arr == c
        es, p = order_e[m], pos[m]
        sval_flat[c, p] = slot_e[es].astype(np.float32)
        for li, k in enumerate(ks):
            idx_flat[li, c, p] = (sg[es] // k).astype(np.int16)
            sub_flat[li, c, p] = (sg[es] % k).astype(np.float32)

    def wrap16(a):  # [S] -> [128, S//16]
        return np.tile(a.reshape(S // 16, 16).T, (8, 1)).copy()

    def wrap128(a):  # [S] -> [128, S//128]
        return a.reshape(S // 128, 128).T.astype(ml_dtypes.bfloat16)

    idx_tbl = np.stack([[wrap16(idx_flat[li, c]) for c in range(NCORES)]
                        for li in range(3)])
    sub_tbl = np.stack([[wrap128(sub_flat[li, c]) for c in range(NCORES)]
                        for li in range(3)])
    sval_tbl = np.stack([wrap128(sval_flat[c]) for c in range(NCORES)])

    x_perm = np.zeros((Npad, x.shape[1]), np.float32)
    x_perm[gid] = x
    dinv_all = dinv_pad.reshape(128, Npad // 128).astype(ml_dtypes.bfloat16)
    dinv_my = dinv_pad.reshape(NCORES, 128, TT).copy()
    dinv2_my = (dinv_my * dinv_my).copy()
    x_my = x_perm.reshape(NCORES, 128, TT * x.shape[1]).copy()    # [8,128,TT*8]

    cnt = np.bincount(batch, minlength=num_graphs).astype(np.float64)
    invc = (1.0 / np.maximum(cnt, 1.0)).astype(np.float64)
    bd = batch[edge_dst].astype(np.int64)
    w = dinv[edge_src].astype(np.float64) * dinv[edge_dst] * invc[bd]
    Pacc = np.bincount(sg * num_graphs + bd, weights=w,
                       minlength=Npad * num_graphs)
    Pacc += np.bincount(gid * num_graphs + batch.astype(np.int64),
                        weights=(dinv.astype(np.float64) ** 2) * invc[batch],
                        minlength=Npad * num_graphs)
    Pacc = Pacc.reshape(Npad, num_graphs).astype(np.float32)
    P_my = Pacc.reshape(NCORES, 128, TT * num_graphs).astype(ml_dtypes.bfloat16)

    x_perm = x_perm.astype(ml_dtypes.bfloat16)
    return dict(prefix=prefix, S=S, Npad=Npad, idx_tbl=idx_tbl, sub_tbl=sub_tbl,
                sval_tbl=sval_tbl, x_perm=x_perm, x_my=x_my,
                dinv_all=dinv_all, dinv_my=dinv_my, dinv2_my=dinv2_my,
                P_my=P_my)


# ------------------------------------------------------------------ device IR
def _build(meta, num_graphs, n_classes, alphas, n_cores=NCORES):
    S, Npad = meta["S"], meta["Npad"]
    prefix = [int(v) for v in meta["prefix"]]
    SH0 = prefix[TT // 2]
    Fs = [8, 16, 32]          # aggregation widths, layers 1-3
    Fos = [16, 32, 64]        # output widths, layers 1-3
    ks = [16, 8, 4]           # nodes per 256B gather row
    nodes_my = NPC

    nc = bacc.Bacc("TRN2", target_bir_lowering=False, debug=False,
                   num_devices=n_cores, num_swdge_queues=4)
    rg = [list(range(n_cores))]

    x_in = nc.dram_tensor("x_perm", [Npad, 8], BF16, kind="ExternalInput")
    xmy_in = nc.dram_tensor("x_my", [128, TT * 8], F32, kind="ExternalInput")
    dall_in = nc.dram_tensor("dinv_all", [128, Npad // 128], BF16,
                             kind="ExternalInput")
    dmy_in = nc.dram_tensor("dinv_my", [128, TT], F32, kind="ExternalInput")
    d2my_in = nc.dram_tensor("dinv2_my", [128, TT], F32, kind="ExternalInput")
    idx_in = [nc.dram_tensor(f"idx{l+1}", [128, S // 16], I16,
                             kind="ExternalInput") for l in range(3)]
    sub_in = [nc.dram_tensor(f"sub{l+1}", [128, S // 128], BF16,
                             kind="ExternalInput") for l in range(3)]
    sval_in = nc.dram_tensor("sval", [128, S // 128], BF16, kind="ExternalInput")
    P_in = nc.dram_tensor("P_my", [128, TT * num_graphs], BF16,
                          kind="ExternalInput")
    W_in = [nc.dram_tensor(f"W{l+1}", [Fs[l], Fos[l]], BF16,
                           kind="ExternalInput") for l in range(3)]
    b_in = [nc.dram_tensor(f"b{l+1}", [Fos[l], 1], F32, kind="ExternalInput")
            for l in range(3)]
    bn_in = [nc.dram_tensor(f"bn{l+1}", [Fos[l], 1], F32, kind="ExternalInput")
             for l in range(3)]
    W4_in = nc.dram_tensor("W4", [64, 128], F32, kind="ExternalInput")
    b4_in = nc.dram_tensor("b4", [128, 1], F32, kind="ExternalInput")
    Wlin_in = nc.dram_tensor("Wlin", [128, n_classes], F32, kind="ExternalInput")
    blin_in = nc.dram_tensor("blin_rep", [num_graphs, n_classes], F32,
                             kind="ExternalInput")
    out_t = nc.dram_tensor("out", [num_graphs, n_classes], F32,
                           kind="ExternalOutput")

    g1 = nc.dram_tensor("g1", [Npad, 8], BF16)
    g2 = nc.dram_tensor("g2", [Npad, 16], BF16)
    g3 = nc.dram_tensor("g3", [Npad, 32], BF16)
    gs = [g1, g2, g3]
    hs2 = nc.dram_tensor("hs2", [nodes_my, 16], BF16)
    hs3 = nc.dram_tensor("hs3", [nodes_my, 32], BF16)
    hss = [hs2, hs3]
    pooled_d = nc.dram_tensor("pooled", [128, num_graphs], F32)
    pooled_r = nc.dram_tensor("pooled_red", [128, num_graphs], F32)

    with tile.TileContext(nc) as tc:
        with (
            tc.tile_pool(name="const", bufs=1) as cpool,
            tc.tile_pool(name="meta", bufs=2) as mpool,
            tc.tile_pool(name="gat", bufs=6) as gpool,
            tc.tile_pool(name="am", bufs=6) as apool,
            tc.tile_pool(name="red", bufs=4) as rpool,
            tc.tile_pool(name="big", bufs=1) as bpool,
            tc.tile_pool(name="ps", bufs=1, space="PSUM") as pspool,
        ):
            nc.gpsimd.load_library(mlp_lib)

            iden = cpool.tile([128, 128], BF16)
            make_identity(nc, iden[:])
            iota = cpool.tile([128, 128], BF16)
            nc.gpsimd.iota(iota[:], [[1, 128]], channel_multiplier=0,
                           allow_small_or_imprecise_dtypes=True)

            dinv_my = cpool.tile([128, TT], F32)
            nc.sync.dma_start(dinv_my[:], dmy_in.ap())
            dinv2_my = cpool.tile([128, TT], F32)
            nc.sync.dma_start(dinv2_my[:], d2my_in.ap())
            sval_sb = cpool.tile([128, S // 128], BF16)
            nc.sync.dma_start(sval_sb[:], sval_in.ap())
            x_my = cpool.tile([128, TT * 8], F32)
            nc.sync.dma_start(x_my[:], xmy_in.ap())
            P_sb = cpool.tile([128, TT * num_graphs], BF16)
            nc.sync.dma_start(P_sb[:], P_in.ap())

            Wt, btl, bntl = [], [], []
            for l in range(3):
                w = cpool.tile([128, Fos[l]], BF16, tag=f"W{l}")
                nc.sync.dma_start(w[:Fs[l], :], W_in[l].ap())
                Wt.append(w)
                b = cpool.tile([128, 1], F32, tag=f"b{l}")
                nc.sync.dma_start(b[:Fos[l], :], b_in[l].ap())
                btl.append(b)
                bn = cpool.tile([128, 1], F32, tag=f"bn{l}")
                nc.sync.dma_start(bn[:Fos[l], :], bn_in[l].ap())
                bntl.append(bn)
            W4_sb = cpool.tile([128, 128], F32, tag="W4")
            nc.sync.dma_start(W4_sb[:64, :], W4_in.ap())
            b4_sb = cpool.tile([128, 1], F32, tag="b4")
            nc.sync.dma_start(b4_sb[:], b4_in.ap())
            Wlin_sb = cpool.tile([128, n_classes], F32, tag="wlin")
            nc.sync.dma_start(Wlin_sb[:], Wlin_in.ap())
            blin_sb = cpool.tile([num_graphs, n_classes], F32, tag="blin")
            nc.sync.dma_start(blin_sb[:], blin_in.ap())

            # ---------------- g1 = dinv * x (full, every core), bf16 packed
            ncols = Npad // 128
            CCH = 128
            xv = x_in.ap().rearrange("(p c) f -> p c f", p=128)
            g1v = g1.ap().rearrange("(p c) f -> p c f", p=128)
            for c0 in range(0, ncols, CCH):
                xt = mpool.tile([128, CCH, 8], BF16, tag="xt")
                nc.sync.dma_start(xt[:], xv[:, c0:c0 + CCH, :])
                da = mpool.tile([128, CCH], BF16, tag="da")
                nc.sync.dma_start(da[:], dall_in.ap()[:, c0:c0 + CCH])
                gt = mpool.tile([128, CCH, 8], BF16, tag="gt")
                nc.vector.tensor_tensor(
                    gt[:], xt[:],
                    da[:, :, None].broadcast_to([128, CCH, 8]),
                    op=mybir.AluOpType.mult)
                nc.sync.dma_start(g1v[:, c0:c0 + CCH, :], gt[:])

            gq = [0]
            aggT = bpool.tile([128, nodes_my // 2], BF16, tag="aggT")
            h_sbT = bpool.tile([128, nodes_my // 2], BF16, tag="h_sbT")
            agg = bpool.tile([128, TT * 32], BF16, tag="agg")
            gnext = bpool.tile([128, TT * 32], BF16, tag="gnext")
            h_resA = bpool.tile([128, TT * 64], BF16, tag="h_resA")
            h_resB = bpool.tile([128, TT * 32], BF16, tag="h_resB")
            IH = max(SH0, S - SH0) // 16
            idx_h = [bpool.tile([128, IH], I16, tag=f"idx_h{h}",
                                name=f"idx_h{h}") for h in range(2)]
            sub_d = [bpool.tile([128, S // 128], BF16, tag=f"sub_d{h}",
                                name=f"sub_d{h}") for h in range(2)]
            nc.sync.dma_start(idx_h[0][:, :SH0 // 16],
                              idx_in[0].ap()[:, :SH0 // 16])
            nc.sync.dma_start(idx_h[1][:, :(S - SH0) // 16],
                              idx_in[0].ap()[:, SH0 // 16:])
            nc.sync.dma_start(sub_d[0][:], sub_in[0].ap())
            pp_sb = cpool.tile([128, num_graphs], F32, tag="pp_sb")
            nc.vector.memset(pp_sb[:], 0.0)

            for li in range(3):
                F, Fo, k = Fs[li], Fos[li], ks[li]
                h_res_prev = [None, h_resA, h_resB][li]
                h_res_next = [h_resA, h_resB, h_resA][li]
                sub_sb = sub_d[li % 2]
                gsrc = gs[li].ap().rearrange("(r k) f -> r (k f)", k=k)

                # ---- aggregation + post-phase, interleaved per node-half.
                # Flat 1024-idx gather stream; groups map to dst tiles via
                # position (tile t spans [t*L, (t+1)*L), L % 128 == 0).
                a_f = alphas[li]
                HT = TT // 2
                psums = {}

                def finish_tile(t, k=k, F=F, li=li,
                                h_res_prev=h_res_prev):
                    # drain psum -> sbuf, slice-reduce k blocks, self-loop
                    psum = psums.pop(t)
                    red0 = rpool.tile([128, 128], F32, tag="red0")
                    nc.scalar.copy(red0[:], psum[:, :128])
                    cur = red0[:].rearrange("p (k f) -> p k f", k=k)
                    kk = k
                    while kk > 1:
                        half = kk // 2
                        nxt = rpool.tile([128, 8, 32], F32, tag=f"red{kk}")
                        nc.vector.tensor_tensor(
                            nxt[:, :half, :F], cur[:, :half, :],
                            cur[:, half:kk, :], op=mybir.AluOpType.add)
                        cur = nxt[:, :half, :F]
                        kk = half
                    sc = rpool.tile([128, 32], F32, tag="sc")
                    hp_src = x_my if li == 0 else h_res_prev
                    nc.vector.tensor_tensor(
                        sc[:, :F], hp_src[:, t * F:(t + 1) * F],
                        dinv2_my[:, t:t + 1].broadcast_to([128, F]),
                        op=mybir.AluOpType.mult)
                    t2 = rpool.tile([128, 32], F32, tag="t2")
                    nc.scalar.activation(
                        t2[:, :F], cur[:, 0, :],
                        mybir.ActivationFunctionType.Identity,
                        scale=dinv_my[:, t:t + 1])
                    nc.vector.tensor_tensor(
                        agg[:, t * F:(t + 1) * F], t2[:, :F], sc[:, :F],
                        op=mybir.AluOpType.add)

                cur_t = [0]
                for hb in range(2):
                  if True:
                    idx_sb = idx_h[hb]
                    hb0 = 0 if hb == 0 else SH0
                    hb1 = SH0 if hb == 0 else S
                    for base in range(hb0, hb1, GIDX):
                        nidx = GIDX
                        ng = nidx // 128
                        gt = gpool.tile([128, 8, 128], BF16, tag="gtile")
                        nc.gpsimd.dma_gather(
                            gt[:, :ng, :], gsrc,
                            idx_sb[:, (base - hb0) // 16:
                                   (base - hb0 + nidx) // 16],
                            nidx, nidx, 128, queue_num=gq[0] % 4)
                        gq[0] += 1
                        sv = sval_sb[:, base // 128:(base + nidx) // 128]
                        sb = sub_sb[:, base // 128:(base + nidx) // 128]
                        A = apool.tile([128, 8, 128], BF16, tag="A")
                        nc.vector.tensor_tensor(
                            A[:, :ng, :],
                            sv[:, :, None].broadcast_to([128, ng, 128]),
                            iota[:, None, :].broadcast_to([128, ng, 128]),
                            op=mybir.AluOpType.is_equal)
                        Mt = apool.tile([128, 128], BF16, tag="M")
                        nc.vector.tensor_tensor(
                            Mt[:, :ng * k].rearrange("p (g k) -> p g k", k=k),
                            sb[:, :, None].broadcast_to([128, ng, k]),
                            iota[:, None, :k].broadcast_to([128, ng, k]),
                            op=mybir.AluOpType.is_equal)
                        Gm = gpool.tile([128, 8, 128], BF16, tag="Gm")
                        nc.vector.tensor_tensor(
                            Gm[:, :ng, :].rearrange("p g (k f) -> p (g k) f",
                                                    k=k),
                            gt[:, :ng, :].rearrange("p g (k f) -> p (g k) f",
                                                    k=k),
                            Mt[:, :ng * k, None].broadcast_to([128, ng * k, F]),
                            op=mybir.AluOpType.mult)
                        for gg in range(ng):
                            pos = base + gg * 128
                            while pos >= prefix[cur_t[0] + 1]:
                                cur_t[0] += 1
                            t = cur_t[0]
                            if t not in psums:
                                psums[t] = pspool.tile(
                                    [128, 512], F32, tag=f"ps{t % 6}",
                                    name=f"pst{t % 6}")
                            nc.tensor.matmul(
                                psums[t][:, :128], A[:, gg, :], Gm[:, gg, :],
                                start=(pos == prefix[t]),
                                stop=(pos == prefix[t + 1] - 128),
                                skip_group_check=True)
                            if pos == prefix[t + 1] - 128:
                                finish_tile(t)
                    if li < 2:
                        nc.sync.dma_start(
                            idx_h[hb][:, :(hb1 - hb0) // 16],
                            idx_in[li + 1].ap()[:, hb0 // 16:hb1 // 16])
                        if hb == 0:
                            nc.sync.dma_start(sub_d[(li + 1) % 2][:],
                                              sub_in[li + 1].ap())

                  # ---- this half: transpose agg -> aggT, W matmul+PReLU,
                  #      transpose back, keep h_res, emit gnext / P-psum
                  if True:
                    t0 = hb * HT
                    for tt in range(HT):
                        t = t0 + tt
                        tp = pspool.tile([128, 512], BF16, tag=f"ps{6 + tt % 2}")
                        nc.tensor.matmul(tp[:F, :128],
                                         agg[:, t * F:(t + 1) * F],
                                         iden[:], is_transpose=True,
                                         skip_group_check=True)
                        nc.scalar.copy(aggT[:F, tt * 128:(tt + 1) * 128],
                                       tp[:F, :128])
                    for n0 in range(0, nodes_my // 2, 512):
                        hp = pspool.tile([128, 512], F32,
                                         tag=f"ps{6 + (n0 // 512) % 2}")
                        nc.tensor.matmul(hp[:Fo, :512], Wt[li][:F, :Fo],
                                         aggT[:F, n0:n0 + 512],
                                         skip_group_check=True)
                        # prelu(x+b) = relu(x+b) - a * relu(-x-b)
                        nc.scalar.activation(
                            h_sbT[:Fo, n0:n0 + 512], hp[:Fo, :512],
                            mybir.ActivationFunctionType.Relu,
                            bias=btl[li][:Fo, :], scale=1.0)
                        hrelu = mpool.tile([128, 512], BF16, tag="hrelu")
                        nc.scalar.activation(
                            hrelu[:Fo, :512], hp[:Fo, :512],
                            mybir.ActivationFunctionType.Relu,
                            bias=bntl[li][:Fo, :], scale=-1.0)
                        nc.vector.scalar_tensor_tensor(
                            h_sbT[:Fo, n0:n0 + 512], hrelu[:Fo, :512],
                            float(-a_f), h_sbT[:Fo, n0:n0 + 512],
                            op0=mybir.AluOpType.mult, op1=mybir.AluOpType.add)
                    for tt in range(HT):
                        t = t0 + tt
                        tb = pspool.tile([128, 512], BF16, tag=f"ps{6 + tt % 2}")
                        nc.tensor.matmul(tb[:128, :Fo],
                                         h_sbT[:Fo, tt * 128:(tt + 1) * 128],
                                         iden[:Fo, :Fo], is_transpose=True,
                                         skip_group_check=True)
                        nc.vector.tensor_copy(
                            h_res_next[:, t * Fo:(t + 1) * Fo], tb[:, :Fo])
                        if li < 2:
                            nc.scalar.activation(
                                gnext[:, t * Fo:(t + 1) * Fo], tb[:, :Fo],
                                mybir.ActivationFunctionType.Identity,
                                scale=dinv_my[:, t:t + 1])
                        else:
                            ppp = pspool.tile([128, 512], F32,
                                              tag=f"ps{6 + (tt + 1) % 2}")
                            nc.tensor.matmul(
                                ppp[:64, :num_graphs],
                                h_res_next[:, t * Fo:(t + 1) * Fo],
                                P_sb[:, t * num_graphs:(t + 1) * num_graphs],
                                start=True, stop=True,
                                skip_group_check=True)
                            nc.vector.tensor_tensor(
                                pp_sb[:64, :], pp_sb[:64, :],
                                ppp[:64, :num_graphs],
                                op=mybir.AluOpType.add)

                    if li < 2:
                        hsv = hss[li].ap().rearrange("(p t) f -> p t f", p=128)
                        nc.sync.dma_start(
                            hsv[:, t0:t0 + HT, :],
                            gnext[:, t0 * Fo:(t0 + HT) * Fo].rearrange(
                                "p (t f) -> p t f", f=Fo))
                if li < 2:
                    if n_cores > 1:
                        nc.gpsimd.collective_compute(
                            "AllGather", mybir.AluOpType.bypass, rg,
                            [hss[li].ap()], [gs[li + 1].ap()])
                    else:
                        nc.sync.dma_start(gs[li + 1].ap()[:nodes_my, :],
                                          hss[li].ap())

            # ---------------- tail: pooledT = W4^T @ ppT + b4; AllReduce; lin
            poolp = pspool.tile([128, 512], F32, tag="ps7")
            nc.tensor.matmul(poolp[:128, :num_graphs], W4_sb[:64, :128],
                             pp_sb[:64, :num_graphs], skip_group_check=True)
            poolT_sb = cpool.tile([128, num_graphs], F32, tag="poolT")
            nc.scalar.activation(poolT_sb[:], poolp[:128, :num_graphs],
                                 mybir.ActivationFunctionType.Identity,
                                 bias=b4_sb[:, :], scale=1.0)
            if n_cores > 1:
                nc.sync.dma_start(pooled_d.ap(), poolT_sb[:])
                nc.gpsimd.collective_compute(
                    "AllReduce", mybir.AluOpType.add, rg,
                    [pooled_d.ap()], [pooled_r.ap()])
                poolT2 = cpool.tile([128, num_graphs], F32, tag="poolT2")
                nc.sync.dma_start(poolT2[:], pooled_r.ap())
            else:
                poolT2 = poolT_sb
            fin = pspool.tile([128, 512], F32, tag="ps6")
            nc.tensor.matmul(fin[:num_graphs, :n_classes], poolT2[:],
                             Wlin_sb[:], skip_group_check=True)
            out_sb = cpool.tile([num_graphs, n_classes], F32, tag="outsb")
            nc.vector.tensor_tensor(out_sb[:], fin[:num_graphs, :n_classes],
                                    blin_sb[:], op=mybir.AluOpType.add)
            nc.sync.dma_start(out_t.ap(), out_sb[:])

    nc.compile()
    return nc


# ------------------------------------------------------------------ entry
def kernel(x, edge_src, edge_dst, batch,
           W1, b1, W2, b2, W3, b3, W4, b4,
           a1, a2, a3, Wlin, blin, n_cores=NCORES):
    x = np.asarray(x, dtype=np.float32)
    edge_src = np.asarray(edge_src, dtype=np.int32)
    edge_dst = np.asarray(edge_dst, dtype=np.int32)
    batch = np.asarray(batch, dtype=np.int32)
    Ws = [np.asarray(w, np.float32) for w in (W1, W2, W3, W4)]
    bs = [np.asarray(b, np.float32) for b in (b1, b2, b3, b4)]
    alphas = [float(a1), float(a2), float(a3)]
    Wlin = np.asarray(Wlin, np.float32)
    blin = np.asarray(blin, np.float32)
    NG, NCLS = 64, Wlin.shape[1]

    meta = _preprocess(x, edge_src, edge_dst, batch, NG)
    nc = _build(meta, NG, NCLS, alphas, n_cores)
    in_maps = _in_maps(meta, Ws, bs, Wlin, blin, NG, n_cores)
    res = run_bass_kernel_spmd(nc, in_maps, core_ids=list(range(n_cores)))
    return np.asarray(res.results[0]["out"], dtype=np.float32)


def _in_maps(meta, Ws, bs, Wlin, blin, NG, n_cores=NCORES):
    in_maps = []
    for c in range(n_cores):
        m = dict(
            x_perm=meta["x_perm"],
            x_my=np.ascontiguousarray(meta["x_my"][c]),
            dinv_all=meta["dinv_all"],
            dinv_my=np.ascontiguousarray(meta["dinv_my"][c]),
            dinv2_my=np.ascontiguousarray(meta["dinv2_my"][c]),
            sval=np.asarray(meta["sval_tbl"][c]),
            P_my=np.asarray(meta["P_my"][c]),
            W4=Ws[3].astype(np.float32),
            b4=np.ascontiguousarray(bs[3].reshape(-1, 1)),
            Wlin=Wlin,
            blin_rep=np.tile(blin[None, :], (NG, 1)).astype(np.float32),
        )
        for l in range(3):
            m[f"idx{l+1}"] = np.asarray(meta["idx_tbl"][l, c])
            m[f"sub{l+1}"] = np.asarray(meta["sub_tbl"][l][c])
            m[f"W{l+1}"] = Ws[l].astype(ml_dtypes.bfloat16)
            m[f"b{l+1}"] = np.ascontiguousarray(bs[l].reshape(-1, 1))
            m[f"bn{l+1}"] = np.ascontiguousarray(-bs[l].reshape(-1, 1))
        in_maps.append(m)
    return in_maps


# revision 24
# speedup vs baseline: 1.0386x; 1.0124x over previous
"""GCN forward (4-layer GCNConv + global mean-pool + linear) on 8 TRN2 cores.

Redesign vs. the v1 kernel (dst-tile dma_gather at 256B/edge, 4 layers):
  * Layer 4 + mean-pool are collapsed into a host-built structure matrix
    P[graph, node] (= pool(1/cnt) . A_hat norms): pooled = (P @ h3) @ W4 + b4.
    The widest aggregation (F=64) becomes a tiny dense TensorE contraction.
  * Gather rows are packed: h stored contiguously [Npad, F] bf16 so one 256B
    dma_gather row holds k = 128/F nodes (16/8/4 for F=8/16/32). An int16 row
    index then covers all of Npad -> no src chunking, and per-(tile) padding
    only (to 128) -> ~213k descriptors/layer/core vs 416k before.
  * Per-edge sub-row selection is a DVE mask (sub == iota_k) multiplied into
    the gathered rows; the one-hot scatter matmul then accumulates a full
    [128 dst, 128col] psum whose k F-wide column blocks are slice-reduced.
  * Self-loop term is computed directly as dinv^2 * h per dst tile (DVE),
    never gathered.
  * Aggregation math: agg_d = dinv_d * sum_{s->d} (dinv_s h_s) + dinv_d^2 h_d,
    with dinv_s folded into the stored gather source.

All graph preprocessing (tile packing, index/mask tables, P) is host numpy
and depends only on graph structure (edges/batch/degrees), never on x or W.
"""

import heapq

import numpy as np
import ml_dtypes

import concourse.bacc as bacc
import concourse.mybir as mybir
import concourse.tile as tile
from concourse.bass_utils import run_bass_kernel_spmd
from concourse.library_config import mlp as mlp_lib
from concourse.masks import make_identity

F32 = mybir.dt.float32
BF16 = mybir.dt.bfloat16
I16 = mybir.dt.int16

NCORES = 8
TT = 128              # dst tiles per core
NPC = TT * 128        # dst nodes per core (16384)
GIDX = 1024           # max idxs per dma_gather instruction (SWDGE ring limit)
PAD_S = 255.0         # slot/sub id for padding positions


# ------------------------------------------------------------------ host prep
def _preprocess(x, edge_src, edge_dst, batch, num_graphs):
    N = x.shape[0]
    Npad = NCORES * NPC
    indeg = np.bincount(edge_dst, minlength=N).astype(np.int64)
    deg = indeg + 1  # self loop
    dinv = (1.0 / np.sqrt(deg.astype(np.float64))).astype(np.float32)

    # Two-tier greedy assignment of dst nodes to (core, tile) bins by
    # indegree: tiles 0..123 target just under 12 gather groups (1536 slots),
    # the last 4 tiles per core absorb the excess -> ~5% fewer pad slots.
    order = np.argsort(-indeg, kind="stable")
    nbins = NCORES * TT
    NHI = 4
    cap_lo = 12 * 128 - 10.0
    t_hi = (len(edge_dst) / NCORES - (TT - NHI) * cap_lo) / NHI
    target = np.full(TT, cap_lo)
    target[TT - NHI:] = max(t_hi, cap_lo)
    heap = [(-target[b % TT], b) for b in range(nbins)]
    heapq.heapify(heap)
    counts = np.zeros(nbins, np.int64)
    loads = np.zeros(nbins, np.float64)
    gid = np.empty(N, np.int64)
    for v in order:
        key, b = heapq.heappop(heap)
        c, t = b // TT, b % TT
        gid[v] = c * NPC + counts[b] * TT + t
        counts[b] += 1
        loads[b] += indeg[v]
        if counts[b] < 128:
            heapq.heappush(heap, (loads[b] - target[t], b))
    assert counts.max() <= 128

    dinv_pad = np.ones(Npad, np.float32)
    dinv_pad[gid] = dinv

    sg, dg = gid[edge_src], gid[edge_dst]
    core_e = dg // NPC
    tile_e = dg % TT
    slot_e = (dg % NPC) // TT
    binid = core_e * TT + tile_e
    cnts = np.bincount(binid, minlength=nbins).reshape(NCORES, TT)
    Ls = (-(-cnts.max(axis=0) // 128) * 128).astype(np.int64)
    Ls[TT // 2 - 1] += (-Ls[:TT // 2].sum()) % 1024     # align half streams
    Ls[TT - 1] += (-Ls[TT // 2:].sum()) % 1024
    prefix = np.concatenate([[0], np.cumsum(Ls)])
    S = int(prefix[-1])

    ks = (16, 8, 4)
    order_e = np.argsort(binid, kind="stable")
    sortedbin = binid[order_e]
    run_start = np.searchsorted(sortedbin, np.arange(nbins))
    rank = np.arange(len(order_e)) - run_start[sortedbin]
    pos = prefix[sortedbin % TT] + rank
    core_arr = sortedbin // TT

    idx_flat = np.zeros((3, NCORES, S), np.int16)
    sub_flat = np.full((3, NCORES, S), PAD_S, np.float32)
    sval_flat = np.full((NCORES, S), PAD_S, np.float32)
    for c in range(NCORES):
        m = core_arr == c
        es, p = order_e[m], pos[m]
        sval_flat[c, p] = slot_e[es].astype(np.float32)
        for li, k in enumerate(ks):
            idx_flat[li, c, p] = (sg[es] // k).astype(np.int16)
            sub_flat[li, c, p] = (sg[es] % k).astype(np.float32)

    def wrap16(a):  # [S] -> [128, S//16]
        return np.tile(a.reshape(S // 16, 16).T, (8, 1)).copy()

    def wrap128(a):  # [S] -> [128, S//128]
        return a.reshape(S // 128, 128).T.astype(ml_dtypes.bfloat16)

    idx_tbl = np.stack([[wrap16(idx_flat[li, c]) for c in range(NCORES)]
                        for li in range(3)])
    sub_tbl = np.stack([[wrap128(sub_flat[li, c]) for c in range(NCORES)]
                        for li in range(3)])
    sval_tbl = np.stack([wrap128(sval_flat[c]) for c in range(NCORES)])

    x_perm = np.zeros((Npad, x.shape[1]), np.float32)
    x_perm[gid] = x
    dinv_all = dinv_pad.reshape(128, Npad // 128).astype(ml_dtypes.bfloat16)
    dinv_my = dinv_pad.reshape(NCORES, 128, TT).copy()
    dinv2_my = (dinv_my * dinv_my).copy()
    x_my = x_perm.reshape(NCORES, 128, TT * x.shape[1]).copy()    # [8,128,TT*8]

    cnt = np.bincount(batch, minlength=num_graphs).astype(np.float64)
    invc = (1.0 / np.maximum(cnt, 1.0)).astype(np.float64)
    bd = batch[edge_dst].astype(np.int64)
    w = dinv[edge_src].astype(np.float64) * dinv[edge_dst] * invc[bd]
    Pacc = np.bincount(sg * num_graphs + bd, weights=w,
                       minlength=Npad * num_graphs)
    Pacc += np.bincount(gid * num_graphs + batch.astype(np.int64),
                        weights=(dinv.astype(np.float64) ** 2) * invc[batch],
                        minlength=Npad * num_graphs)
    Pacc = Pacc.reshape(Npad, num_graphs).astype(np.float32)
    P_my = Pacc.reshape(NCORES, 128, TT * num_graphs).astype(ml_dtypes.bfloat16)

    x_perm = x_perm.astype(ml_dtypes.bfloat16)
    return dict(prefix=prefix, S=S, Npad=Npad, idx_tbl=idx_tbl, sub_tbl=sub_tbl,
                sval_tbl=sval_tbl, x_perm=x_perm, x_my=x_my,
                dinv_all=dinv_all, dinv_my=dinv_my, dinv2_my=dinv2_my,
                P_my=P_my)


# ------------------------------------------------------------------ device IR
def _build(meta, num_graphs, n_classes, alphas, n_cores=NCORES):
    S, Npad = meta["S"], meta["Npad"]
    prefix = [int(v) for v in meta["prefix"]]
    SH0 = prefix[TT // 2]
    Fs = [8, 16, 32]          # aggregation widths, layers 1-3
    Fos = [16, 32, 64]        # output widths, layers 1-3
    ks = [16, 8, 4]           # nodes per 256B gather row
    nodes_my = NPC

    nc = bacc.Bacc("TRN2", target_bir_lowering=False, debug=False,
                   num_devices=n_cores, num_swdge_queues=4)
    rg = [list(range(n_cores))]

    x_in = nc.dram_tensor("x_perm", [Npad, 8], BF16, kind="ExternalInput")
    xmy_in = nc.dram_tensor("x_my", [128, TT * 8], F32, kind="ExternalInput")
    dall_in = nc.dram_tensor("dinv_all", [128, Npad // 128], BF16,
                             kind="ExternalInput")
    dmy_in = nc.dram_tensor("dinv_my", [128, TT], F32, kind="ExternalInput")
    d2my_in = nc.dram_tensor("dinv2_my", [128, TT], F32, kind="ExternalInput")
    idx_in = [nc.dram_tensor(f"idx{l+1}", [128, S // 16], I16,
                             kind="ExternalInput") for l in range(3)]
    sub_in = [nc.dram_tensor(f"sub{l+1}", [128, S // 128], BF16,
                             kind="ExternalInput") for l in range(3)]
    sval_in = nc.dram_tensor("sval", [128, S // 128], BF16, kind="ExternalInput")
    P_in = nc.dram_tensor("P_my", [128, TT * num_graphs], BF16,
                          kind="ExternalInput")
    W_in = [nc.dram_tensor(f"W{l+1}", [Fs[l], Fos[l]], BF16,
                           kind="ExternalInput") for l in range(3)]
    b_in = [nc.dram_tensor(f"b{l+1}", [Fos[l], 1], F32, kind="ExternalInput")
            for l in range(3)]
    bn_in = [nc.dram_tensor(f"bn{l+1}", [Fos[l], 1], F32, kind="ExternalInput")
             for l in range(3)]
    W4_in = nc.dram_tensor("W4", [64, 128], F32, kind="ExternalInput")
    b4_in = nc.dram_tensor("b4", [128, 1], F32, kind="ExternalInput")
    Wlin_in = nc.dram_tensor("Wlin", [128, n_classes], F32, kind="ExternalInput")
    blin_in = nc.dram_tensor("blin_rep", [num_graphs, n_classes], F32,
                             kind="ExternalInput")
    out_t = nc.dram_tensor("out", [num_graphs, n_classes], F32,
                           kind="ExternalOutput")

    g1 = nc.dram_tensor("g1", [Npad, 8], BF16)
    g2 = nc.dram_tensor("g2", [Npad, 16], BF16, addr_space="Shared")
    g3 = nc.dram_tensor("g3", [Npad, 32], BF16, addr_space="Shared")
    gs = [g1, g2, g3]
    hs2 = nc.dram_tensor("hs2", [nodes_my, 16], BF16)
    hs3 = nc.dram_tensor("hs3", [nodes_my, 32], BF16)
    hss = [hs2, hs3]
    pooled_d = nc.dram_tensor("pooled", [128, num_graphs], F32)
    pooled_r = nc.dram_tensor("pooled_red", [128, num_graphs], F32, addr_space="Shared")

    with tile.TileContext(nc) as tc:
        with (
            tc.tile_pool(name="const", bufs=1) as cpool,
            tc.tile_pool(name="meta", bufs=2) as mpool,
            tc.tile_pool(name="gat", bufs=6) as gpool,
            tc.tile_pool(name="am", bufs=6) as apool,
            tc.tile_pool(name="red", bufs=4) as rpool,
            tc.tile_pool(name="big", bufs=1) as bpool,
            tc.tile_pool(name="ps", bufs=1, space="PSUM") as pspool,
        ):
            nc.gpsimd.load_library(mlp_lib)

            iden = cpool.tile([128, 128], BF16)
            make_identity(nc, iden[:])
            iota = cpool.tile([128, 128], BF16)
            nc.gpsimd.iota(iota[:], [[1, 128]], channel_multiplier=0,
                           allow_small_or_imprecise_dtypes=True)

            dinv_my = cpool.tile([128, TT], F32)
            nc.sync.dma_start(dinv_my[:], dmy_in.ap())
            dinv2_my = cpool.tile([128, TT], F32)
            nc.sync.dma_start(dinv2_my[:], d2my_in.ap())
            sval_sb = cpool.tile([128, S // 128], BF16)
            nc.sync.dma_start(sval_sb[:], sval_in.ap())
            x_my = cpool.tile([128, TT * 8], F32)
            nc.sync.dma_start(x_my[:], xmy_in.ap())
            P_sb = cpool.tile([128, TT * num_graphs], BF16)
            nc.sync.dma_start(P_sb[:], P_in.ap())

            Wt, btl, bntl = [], [], []
            for l in range(3):
                w = cpool.tile([128, Fos[l]], BF16, tag=f"W{l}")
                nc.sync.dma_start(w[:Fs[l], :], W_in[l].ap())
                Wt.append(w)
                b = cpool.tile([128, 1], F32, tag=f"b{l}")
                nc.sync.dma_start(b[:Fos[l], :], b_in[l].ap())
                btl.append(b)
                bn = cpool.tile([128, 1], F32, tag=f"bn{l}")
                nc.sync.dma_start(bn[:Fos[l], :], bn_in[l].ap())
                bntl.append(bn)
            W4_sb = cpool.tile([128, 128], F32, tag="W4")
            nc.sync.dma_start(W4_sb[:64, :], W4_in.ap())
            b4_sb = cpool.tile([128, 1], F32, tag="b4")
            nc.sync.dma_start(b4_sb[:], b4_in.ap())
            Wlin_sb = cpool.tile([128, n_classes], F32, tag="wlin")
            nc.sync.dma_start(Wlin_sb[:], Wlin_in.ap())
            blin_sb = cpool.tile([num_graphs, n_classes], F32, tag="blin")
            nc.sync.dma_start(blin_sb[:], blin_in.ap())

            # ---------------- g1 = dinv * x (full, every core), bf16 packed
            ncols = Npad // 128
            CCH = 128
            xv = x_in.ap().rearrange("(p c) f -> p c f", p=128)
            g1v = g1.ap().rearrange("(p c) f -> p c f", p=128)
            for c0 in range(0, ncols, CCH):
                xt = mpool.tile([128, CCH, 8], BF16, tag="xt")
                nc.sync.dma_start(xt[:], xv[:, c0:c0 + CCH, :])
                da = mpool.tile([128, CCH], BF16, tag="da")
                nc.sync.dma_start(da[:], dall_in.ap()[:, c0:c0 + CCH])
                gt = mpool.tile([128, CCH, 8], BF16, tag="gt")
                nc.vector.tensor_tensor(
                    gt[:], xt[:],
                    da[:, :, None].broadcast_to([128, CCH, 8]),
                    op=mybir.AluOpType.mult)
                nc.sync.dma_start(g1v[:, c0:c0 + CCH, :], gt[:])

            gq = [0]
            aggT = bpool.tile([128, nodes_my // 2], BF16, tag="aggT")
            h_sbT = bpool.tile([128, nodes_my // 2], BF16, tag="h_sbT")
            agg = bpool.tile([128, TT * 32], BF16, tag="agg")
            gnext = bpool.tile([128, TT * 32], BF16, tag="gnext")
            h_resA = bpool.tile([128, TT * 64], BF16, tag="h_resA")
            h_resB = bpool.tile([128, TT * 32], BF16, tag="h_resB")
            IH = max(SH0, S - SH0) // 16
            idx_h = [bpool.tile([128, IH], I16, tag=f"idx_h{h}",
                                name=f"idx_h{h}") for h in range(2)]
            sub_d = [bpool.tile([128, S // 128], BF16, tag=f"sub_d{h}",
                                name=f"sub_d{h}") for h in range(2)]
            nc.sync.dma_start(idx_h[0][:, :SH0 // 16],
                              idx_in[0].ap()[:, :SH0 // 16])
            nc.sync.dma_start(idx_h[1][:, :(S - SH0) // 16],
                              idx_in[0].ap()[:, SH0 // 16:])
            nc.sync.dma_start(sub_d[0][:], sub_in[0].ap())
            pp_sb = cpool.tile([128, num_graphs], F32, tag="pp_sb")
            nc.vector.memset(pp_sb[:], 0.0)

            for li in range(3):
                F, Fo, k = Fs[li], Fos[li], ks[li]
                h_res_prev = [None, h_resA, h_resB][li]
                h_res_next = [h_resA, h_resB, h_resA][li]
                sub_sb = sub_d[li % 2]
                gsrc = gs[li].ap().rearrange("(r k) f -> r (k f)", k=k)

                # ---- aggregation + post-phase, interleaved per node-half.
                # Flat 1024-idx gather stream; groups map to dst tiles via
                # position (tile t spans [t*L, (t+1)*L), L % 128 == 0).
                a_f = alphas[li]
                HT = TT // 2
                psums = {}

                def finish_tile(t, k=k, F=F, li=li,
                                h_res_prev=h_res_prev):
                    # drain psum -> sbuf, slice-reduce k blocks, self-loop
                    psum = psums.pop(t)
                    red0 = rpool.tile([128, 128], F32, tag="red0")
                    nc.scalar.copy(red0[:], psum[:, :128])
                    cur = red0[:].rearrange("p (k f) -> p k f", k=k)
                    kk = k
                    while kk > 1:
                        half = kk // 2
                        nxt = rpool.tile([128, 8, 32], F32, tag=f"red{kk}")
                        nc.vector.tensor_tensor(
                            nxt[:, :half, :F], cur[:, :half, :],
                            cur[:, half:kk, :], op=mybir.AluOpType.add)
                        cur = nxt[:, :half, :F]
                        kk = half
                    sc = rpool.tile([128, 32], F32, tag="sc")
                    hp_src = x_my if li == 0 else h_res_prev
                    nc.vector.tensor_tensor(
                        sc[:, :F], hp_src[:, t * F:(t + 1) * F],
                        dinv2_my[:, t:t + 1].broadcast_to([128, F]),
                        op=mybir.AluOpType.mult)
                    t2 = rpool.tile([128, 32], F32, tag="t2")
                    nc.scalar.activation(
                        t2[:, :F], cur[:, 0, :],
                        mybir.ActivationFunctionType.Identity,
                        scale=dinv_my[:, t:t + 1])
                    nc.vector.tensor_tensor(
                        agg[:, t * F:(t + 1) * F], t2[:, :F], sc[:, :F],
                        op=mybir.AluOpType.add)

                cur_t = [0]
                for hb in range(2):
                  if True:
                    idx_sb = idx_h[hb]
                    hb0 = 0 if hb == 0 else SH0
                    hb1 = SH0 if hb == 0 else S
                    for base in range(hb0, hb1, GIDX):
                        nidx = GIDX
                        ng = nidx // 128
                        gt = gpool.tile([128, 8, 128], BF16, tag="gtile")
                        nc.gpsimd.dma_gather(
                            gt[:, :ng, :], gsrc,
                            idx_sb[:, (base - hb0) // 16:
                                   (base - hb0 + nidx) // 16],
                            nidx, nidx, 128, queue_num=gq[0] % 4)
                        gq[0] += 1
                        sv = sval_sb[:, base // 128:(base + nidx) // 128]
                        sb = sub_sb[:, base // 128:(base + nidx) // 128]
                        A = apool.tile([128, 8, 128], BF16, tag="A")
                        nc.vector.tensor_tensor(
                            A[:, :ng, :],
                            sv[:, :, None].broadcast_to([128, ng, 128]),
                            iota[:, None, :].broadcast_to([128, ng, 128]),
                            op=mybir.AluOpType.is_equal)
                        Mt = apool.tile([128, 128], BF16, tag="M")
                        nc.vector.tensor_tensor(
                            Mt[:, :ng * k].rearrange("p (g k) -> p g k", k=k),
                            sb[:, :, None].broadcast_to([128, ng, k]),
                            iota[:, None, :k].broadcast_to([128, ng, k]),
                            op=mybir.AluOpType.is_equal)
                        Gm = gpool.tile([128, 8, 128], BF16, tag="Gm")
                        nc.vector.tensor_tensor(
                            Gm[:, :ng, :].rearrange("p g (k f) -> p (g k) f",
                                                    k=k),
                            gt[:, :ng, :].rearrange("p g (k f) -> p (g k) f",
                                                    k=k),
                            Mt[:, :ng * k, None].broadcast_to([128, ng * k, F]),
                            op=mybir.AluOpType.mult)
                        for gg in range(ng):
                            pos = base + gg * 128
                            while pos >= prefix[cur_t[0] + 1]:
                                cur_t[0] += 1
                            t = cur_t[0]
                            if t not in psums:
                                psums[t] = pspool.tile(
                                    [128, 512], F32, tag=f"ps{t % 6}",
                                    name=f"pst{t % 6}")
                            nc.tensor.matmul(
                                psums[t][:, :128], A[:, gg, :], Gm[:, gg, :],
                                start=(pos == prefix[t]),
                                stop=(pos == prefix[t + 1] - 128),
                                skip_group_check=True)
                            if pos == prefix[t + 1] - 128:
                                finish_tile(t)
                    if li < 2:
                        nc.sync.dma_start(
                            idx_h[hb][:, :(hb1 - hb0) // 16],
                            idx_in[li + 1].ap()[:, hb0 // 16:hb1 // 16])
                        if hb == 0:
                            nc.sync.dma_start(sub_d[(li + 1) % 2][:],
                                              sub_in[li + 1].ap())

                  # ---- this half: transpose agg -> aggT, W matmul+PReLU,
                  #      transpose back, keep h_res, emit gnext / P-psum
                  if True:
                    t0 = hb * HT
                    for tt in range(HT):
                        t = t0 + tt
                        tp = pspool.tile([128, 512], BF16, tag=f"ps{6 + tt % 2}")
                        nc.tensor.matmul(tp[:F, :128],
                                         agg[:, t * F:(t + 1) * F],
                                         iden[:], is_transpose=True,
                                         skip_group_check=True)
                        nc.scalar.copy(aggT[:F, tt * 128:(tt + 1) * 128],
                                       tp[:F, :128])
                    for n0 in range(0, nodes_my // 2, 512):
                        hp = pspool.tile([128, 512], F32,
                                         tag=f"ps{6 + (n0 // 512) % 2}")
                        nc.tensor.matmul(hp[:Fo, :512], Wt[li][:F, :Fo],
                                         aggT[:F, n0:n0 + 512],
                                         skip_group_check=True)
                        # prelu(x+b) = relu(x+b) - a * relu(-x-b)
                        nc.scalar.activation(
                            h_sbT[:Fo, n0:n0 + 512], hp[:Fo, :512],
                            mybir.ActivationFunctionType.Relu,
                            bias=btl[li][:Fo, :], scale=1.0)
                        hrelu = mpool.tile([128, 512], BF16, tag="hrelu")
                        nc.scalar.activation(
                            hrelu[:Fo, :512], hp[:Fo, :512],
                            mybir.ActivationFunctionType.Relu,
                            bias=bntl[li][:Fo, :], scale=-1.0)
                        nc.vector.scalar_tensor_tensor(
                            h_sbT[:Fo, n0:n0 + 512], hrelu[:Fo, :512],
                            float(-a_f), h_sbT[:Fo, n0:n0 + 512],
                            op0=mybir.AluOpType.mult, op1=mybir.AluOpType.add)
                    for tt in range(HT):
                        t = t0 + tt
                        tb = pspool.tile([128, 512], BF16, tag=f"ps{6 + tt % 2}")
                        nc.tensor.matmul(tb[:128, :Fo],
                                         h_sbT[:Fo, tt * 128:(tt + 1) * 128],
                                         iden[:Fo, :Fo], is_transpose=True,
                                         skip_group_check=True)
                        nc.vector.tensor_copy(
                            h_res_next[:, t * Fo:(t + 1) * Fo], tb[:, :Fo])
                        if li < 2:
                            nc.scalar.activation(
                                gnext[:, t * Fo:(t + 1) * Fo], tb[:, :Fo],
                                mybir.ActivationFunctionType.Identity,
                                scale=dinv_my[:, t:t + 1])
                        else:
                            ppp = pspool.tile([128, 512], F32,
                                              tag=f"ps{6 + (tt + 1) % 2}")
                            nc.tensor.matmul(
                                ppp[:64, :num_graphs],
                                h_res_next[:, t * Fo:(t + 1) * Fo],
                                P_sb[:, t * num_graphs:(t + 1) * num_graphs],
                                start=True, stop=True,
                                skip_group_check=True)
                            nc.vector.tensor_tensor(
                                pp_sb[:64, :], pp_sb[:64, :],
                                ppp[:64, :num_graphs],
                                op=mybir.AluOpType.add)

                    if li < 2:
                        hsv = hss[li].ap().rearrange("(p t) f -> p t f", p=128)
                        nc.sync.dma_start(
                            hsv[:, t0:t0 + HT, :],
                            gnext[:, t0 * Fo:(t0 + HT) * Fo].rearrange(
                                "p (t f) -> p t f", f=Fo))
                if li < 2:
                    if n_cores > 1:
                        nc.gpsimd.collective_compute(
                            "AllGather", mybir.AluOpType.bypass, rg,
                            [hss[li].ap()], [gs[li + 1].ap()])
                    else:
                        nc.sync.dma_start(gs[li + 1].ap()[:nodes_my, :],
                                          hss[li].ap())

            # ---------------- tail: pooledT = W4^T @ ppT + b4; AllReduce; lin
            poolp = pspool.tile([128, 512], F32, tag="ps7")
            nc.tensor.matmul(poolp[:128, :num_graphs], W4_sb[:64, :128],
                             pp_sb[:64, :num_graphs], skip_group_check=True)
            poolT_sb = cpool.tile([128, num_graphs], F32, tag="poolT")
            nc.scalar.activation(poolT_sb[:], poolp[:128, :num_graphs],
                                 mybir.ActivationFunctionType.Identity,
                                 bias=b4_sb[:, :], scale=1.0)
            if n_cores > 1:
                nc.sync.dma_start(pooled_d.ap(), poolT_sb[:])
                nc.gpsimd.collective_compute(
                    "AllReduce", mybir.AluOpType.add, rg,
                    [pooled_d.ap()], [pooled_r.ap()])
                poolT2 = cpool.tile([128, num_graphs], F32, tag="poolT2")
                nc.sync.dma_start(poolT2[:], pooled_r.ap())
            else:
                poolT2 = poolT_sb
            fin = pspool.tile([128, 512], F32, tag="ps6")
            nc.tensor.matmul(fin[:num_graphs, :n_classes], poolT2[:],
                             Wlin_sb[:], skip_group_check=True)
            out_sb = cpool.tile([num_graphs, n_classes], F32, tag="outsb")
            nc.vector.tensor_tensor(out_sb[:], fin[:num_graphs, :n_classes],
                                    blin_sb[:], op=mybir.AluOpType.add)
            nc.sync.dma_start(out_t.ap(), out_sb[:])

    nc.compile()
    return nc


# ------------------------------------------------------------------ entry
def kernel(x, edge_src, edge_dst, batch,
           W1, b1, W2, b2, W3, b3, W4, b4,
           a1, a2, a3, Wlin, blin, n_cores=NCORES):
    x = np.asarray(x, dtype=np.float32)
    edge_src = np.asarray(edge_src, dtype=np.int32)
    edge_dst = np.asarray(edge_dst, dtype=np.int32)
    batch = np.asarray(batch, dtype=np.int32)
    Ws = [np.asarray(w, np.float32) for w in (W1, W2, W3, W4)]
    bs = [np.asarray(b, np.float32) for b in (b1, b2, b3, b4)]
    alphas = [float(a1), float(a2), float(a3)]
    Wlin = np.asarray(Wlin, np.float32)
    blin = np.asarray(blin, np.float32)
    NG, NCLS = 64, Wlin.shape[1]

    meta = _preprocess(x, edge_src, edge_dst, batch, NG)
    nc = _build(meta, NG, NCLS, alphas, n_cores)
    in_maps = _in_maps(meta, Ws, bs, Wlin, blin, NG, n_cores)
    res = run_bass_kernel_spmd(nc, in_maps, core_ids=list(range(n_cores)))
    return np.asarray(res.results[0]["out"], dtype=np.float32)


def _in_maps(meta, Ws, bs, Wlin, blin, NG, n_cores=NCORES):
    in_maps = []
    for c in range(n_cores):
        m = dict(
            x_perm=meta["x_perm"],
            x_my=np.ascontiguousarray(meta["x_my"][c]),
            dinv_all=meta["dinv_all"],
            dinv_my=np.ascontiguousarray(meta["dinv_my"][c]),
            dinv2_my=np.ascontiguousarray(meta["dinv2_my"][c]),
            sval=np.asarray(meta["sval_tbl"][c]),
            P_my=np.asarray(meta["P_my"][c]),
            W4=Ws[3].astype(np.float32),
            b4=np.ascontiguousarray(bs[3].reshape(-1, 1)),
            Wlin=Wlin,
            blin_rep=np.tile(blin[None, :], (NG, 1)).astype(np.float32),
        )
        for l in range(3):
            m[f"idx{l+1}"] = np.asarray(meta["idx_tbl"][l, c])
            m[f"sub{l+1}"] = np.asarray(meta["sub_tbl"][l][c])
            m[f"W{l+1}"] = Ws[l].astype(ml_dtypes.bfloat16)
            m[f"b{l+1}"] = np.ascontiguousarray(bs[l].reshape(-1, 1))
            m[f"bn{l+1}"] = np.ascontiguousarray(-bs[l].reshape(-1, 1))
        in_maps.append(m)
    return in_maps


# revision 26
# speedup vs baseline: 1.0457x; 1.0068x over previous
"""GCN forward (4-layer GCNConv + global mean-pool + linear) on 8 TRN2 cores.

Redesign vs. the v1 kernel (dst-tile dma_gather at 256B/edge, 4 layers):
  * Layer 4 + mean-pool are collapsed into a host-built structure matrix
    P[graph, node] (= pool(1/cnt) . A_hat norms): pooled = (P @ h3) @ W4 + b4.
    The widest aggregation (F=64) becomes a tiny dense TensorE contraction.
  * Gather rows are packed: h stored contiguously [Npad, F] bf16 so one 256B
    dma_gather row holds k = 128/F nodes (16/8/4 for F=8/16/32). An int16 row
    index then covers all of Npad -> no src chunking, and per-(tile) padding
    only (to 128) -> ~213k descriptors/layer/core vs 416k before.
  * Per-edge sub-row selection is a DVE mask (sub == iota_k) multiplied into
    the gathered rows; the one-hot scatter matmul then accumulates a full
    [128 dst, 128col] psum whose k F-wide column blocks are slice-reduced.
  * Self-loop term is computed directly as dinv^2 * h per dst tile (DVE),
    never gathered.
  * Aggregation math: agg_d = dinv_d * sum_{s->d} (dinv_s h_s) + dinv_d^2 h_d,
    with dinv_s folded into the stored gather source.

All graph preprocessing (tile packing, index/mask tables, P) is host numpy
and depends only on graph structure (edges/batch/degrees), never on x or W.
"""

import heapq

import numpy as np
import ml_dtypes

import concourse.bacc as bacc
import concourse.mybir as mybir
import concourse.tile as tile
from concourse.bass_utils import run_bass_kernel_spmd
from concourse.library_config import mlp as mlp_lib
from concourse.masks import make_identity

F32 = mybir.dt.float32
BF16 = mybir.dt.bfloat16
I16 = mybir.dt.int16

NCORES = 8
TT = 128              # dst tiles per core
NPC = TT * 128        # dst nodes per core (16384)
GIDX = 1024           # max idxs per dma_gather instruction (SWDGE ring limit)
PAD_S = 255.0         # slot/sub id for padding positions


# ------------------------------------------------------------------ host prep
def _preprocess(x, edge_src, edge_dst, batch, num_graphs):
    N = x.shape[0]
    Npad = NCORES * NPC
    indeg = np.bincount(edge_dst, minlength=N).astype(np.int64)
    deg = indeg + 1  # self loop
    dinv = (1.0 / np.sqrt(deg.astype(np.float64))).astype(np.float32)

    # Two-tier greedy assignment of dst nodes to (core, tile) bins by
    # indegree: tiles 0..123 target just under 12 gather groups (1536 slots),
    # the last 4 tiles per core absorb the excess -> ~5% fewer pad slots.
    order = np.argsort(-indeg, kind="stable")
    nbins = NCORES * TT
    NHI = 4
    cap_lo = 12 * 128 - 10.0
    t_hi = (len(edge_dst) / NCORES - (TT - NHI) * cap_lo) / NHI
    target = np.full(TT, cap_lo)
    for b_hi in (0, 1, TT // 2, TT // 2 + 1):
        target[b_hi] = max(t_hi, cap_lo)
    heap = [(-target[b % TT], b) for b in range(nbins)]
    heapq.heapify(heap)
    counts = np.zeros(nbins, np.int64)
    loads = np.zeros(nbins, np.float64)
    gid = np.empty(N, np.int64)
    for v in order:
        key, b = heapq.heappop(heap)
        c, t = b // TT, b % TT
        gid[v] = c * NPC + counts[b] * TT + t
        counts[b] += 1
        loads[b] += indeg[v]
        if counts[b] < 128:
            heapq.heappush(heap, (loads[b] - target[t], b))
    assert counts.max() <= 128

    dinv_pad = np.ones(Npad, np.float32)
    dinv_pad[gid] = dinv

    sg, dg = gid[edge_src], gid[edge_dst]
    core_e = dg // NPC
    tile_e = dg % TT
    slot_e = (dg % NPC) // TT
    binid = core_e * TT + tile_e
    cnts = np.bincount(binid, minlength=nbins).reshape(NCORES, TT)
    Ls = (-(-cnts.max(axis=0) // 128) * 128).astype(np.int64)
    Ls[TT // 2 - 1] += (-Ls[:TT // 2].sum()) % 1024     # align half streams
    Ls[TT - 1] += (-Ls[TT // 2:].sum()) % 1024
    prefix = np.concatenate([[0], np.cumsum(Ls)])
    S = int(prefix[-1])

    ks = (16, 8, 4)
    order_e = np.argsort(binid, kind="stable")
    sortedbin = binid[order_e]
    run_start = np.searchsorted(sortedbin, np.arange(nbins))
    rank = np.arange(len(order_e)) - run_start[sortedbin]
    pos = prefix[sortedbin % TT] + rank
    core_arr = sortedbin // TT

    idx_flat = np.zeros((3, NCORES, S), np.int16)
    sub_flat = np.full((3, NCORES, S), PAD_S, np.float32)
    sval_flat = np.full((NCORES, S), PAD_S, np.float32)
    for c in range(NCORES):
        m = core_arr == c
        es, p = order_e[m], pos[m]
        sval_flat[c, p] = slot_e[es].astype(np.float32)
        for li, k in enumerate(ks):
            idx_flat[li, c, p] = (sg[es] // k).astype(np.int16)
            sub_flat[li, c, p] = (sg[es] % k).astype(np.float32)

    def wrap16(a):  # [S] -> [128, S//16]
        return np.tile(a.reshape(S // 16, 16).T, (8, 1)).copy()

    def wrap128(a):  # [S] -> [128, S//128]
        return a.reshape(S // 128, 128).T.astype(ml_dtypes.bfloat16)

    idx_tbl = np.stack([[wrap16(idx_flat[li, c]) for c in range(NCORES)]
                        for li in range(3)])
    sub_tbl = np.stack([[wrap128(sub_flat[li, c]) for c in range(NCORES)]
                        for li in range(3)])
    sval_tbl = np.stack([wrap128(sval_flat[c]) for c in range(NCORES)])

    x_perm = np.zeros((Npad, x.shape[1]), np.float32)
    x_perm[gid] = x
    dinv_all = dinv_pad.reshape(128, Npad // 128).astype(ml_dtypes.bfloat16)
    dinv_my = dinv_pad.reshape(NCORES, 128, TT).copy()
    dinv2_my = (dinv_my * dinv_my).copy()
    x_my = x_perm.reshape(NCORES, 128, TT * x.shape[1]).copy()    # [8,128,TT*8]

    cnt = np.bincount(batch, minlength=num_graphs).astype(np.float64)
    invc = (1.0 / np.maximum(cnt, 1.0)).astype(np.float64)
    bd = batch[edge_dst].astype(np.int64)
    w = dinv[edge_src].astype(np.float64) * dinv[edge_dst] * invc[bd]
    Pacc = np.bincount(sg * num_graphs + bd, weights=w,
                       minlength=Npad * num_graphs)
    Pacc += np.bincount(gid * num_graphs + batch.astype(np.int64),
                        weights=(dinv.astype(np.float64) ** 2) * invc[batch],
                        minlength=Npad * num_graphs)
    Pacc = Pacc.reshape(Npad, num_graphs).astype(np.float32)
    P_my = Pacc.reshape(NCORES, 128, TT * num_graphs).astype(ml_dtypes.bfloat16)

    x_perm = x_perm.astype(ml_dtypes.bfloat16)
    return dict(prefix=prefix, S=S, Npad=Npad, idx_tbl=idx_tbl, sub_tbl=sub_tbl,
                sval_tbl=sval_tbl, x_perm=x_perm, x_my=x_my,
                dinv_all=dinv_all, dinv_my=dinv_my, dinv2_my=dinv2_my,
                P_my=P_my)


# ------------------------------------------------------------------ device IR
def _build(meta, num_graphs, n_classes, alphas, n_cores=NCORES):
    S, Npad = meta["S"], meta["Npad"]
    prefix = [int(v) for v in meta["prefix"]]
    SH0 = prefix[TT // 2]
    Fs = [8, 16, 32]          # aggregation widths, layers 1-3
    Fos = [16, 32, 64]        # output widths, layers 1-3
    ks = [16, 8, 4]           # nodes per 256B gather row
    nodes_my = NPC

    nc = bacc.Bacc("TRN2", target_bir_lowering=False, debug=False,
                   num_devices=n_cores, num_swdge_queues=4)
    rg = [list(range(n_cores))]

    x_in = nc.dram_tensor("x_perm", [Npad, 8], BF16, kind="ExternalInput")
    xmy_in = nc.dram_tensor("x_my", [128, TT * 8], F32, kind="ExternalInput")
    dall_in = nc.dram_tensor("dinv_all", [128, Npad // 128], BF16,
                             kind="ExternalInput")
    dmy_in = nc.dram_tensor("dinv_my", [128, TT], F32, kind="ExternalInput")
    d2my_in = nc.dram_tensor("dinv2_my", [128, TT], F32, kind="ExternalInput")
    idx_in = [nc.dram_tensor(f"idx{l+1}", [128, S // 16], I16,
                             kind="ExternalInput") for l in range(3)]
    sub_in = [nc.dram_tensor(f"sub{l+1}", [128, S // 128], BF16,
                             kind="ExternalInput") for l in range(3)]
    sval_in = nc.dram_tensor("sval", [128, S // 128], BF16, kind="ExternalInput")
    P_in = nc.dram_tensor("P_my", [128, TT * num_graphs], BF16,
                          kind="ExternalInput")
    W_in = [nc.dram_tensor(f"W{l+1}", [Fs[l], Fos[l]], BF16,
                           kind="ExternalInput") for l in range(3)]
    b_in = [nc.dram_tensor(f"b{l+1}", [Fos[l], 1], F32, kind="ExternalInput")
            for l in range(3)]
    bn_in = [nc.dram_tensor(f"bn{l+1}", [Fos[l], 1], F32, kind="ExternalInput")
             for l in range(3)]
    W4_in = nc.dram_tensor("W4", [64, 128], F32, kind="ExternalInput")
    b4_in = nc.dram_tensor("b4", [128, 1], F32, kind="ExternalInput")
    Wlin_in = nc.dram_tensor("Wlin", [128, n_classes], F32, kind="ExternalInput")
    blin_in = nc.dram_tensor("blin_rep", [num_graphs, n_classes], F32,
                             kind="ExternalInput")
    out_t = nc.dram_tensor("out", [num_graphs, n_classes], F32,
                           kind="ExternalOutput")

    g1 = nc.dram_tensor("g1", [Npad, 8], BF16)
    g2 = nc.dram_tensor("g2", [Npad, 16], BF16, addr_space="Shared")
    g3 = nc.dram_tensor("g3", [Npad, 32], BF16, addr_space="Shared")
    gs = [g1, g2, g3]
    hs2 = nc.dram_tensor("hs2", [nodes_my, 16], BF16)
    hs3 = nc.dram_tensor("hs3", [nodes_my, 32], BF16)
    hss = [hs2, hs3]
    pooled_d = nc.dram_tensor("pooled", [128, num_graphs], F32)
    pooled_r = nc.dram_tensor("pooled_red", [128, num_graphs], F32, addr_space="Shared")

    with tile.TileContext(nc) as tc:
        with (
            tc.tile_pool(name="const", bufs=1) as cpool,
            tc.tile_pool(name="meta", bufs=2) as mpool,
            tc.tile_pool(name="gat", bufs=6) as gpool,
            tc.tile_pool(name="am", bufs=6) as apool,
            tc.tile_pool(name="red", bufs=3) as rpool,
            tc.tile_pool(name="big", bufs=1) as bpool,
            tc.tile_pool(name="ps", bufs=1, space="PSUM") as pspool,
        ):
            nc.gpsimd.load_library(mlp_lib)

            iden = cpool.tile([128, 128], BF16)
            make_identity(nc, iden[:])
            iota = cpool.tile([128, 128], BF16)
            nc.gpsimd.iota(iota[:], [[1, 128]], channel_multiplier=0,
                           allow_small_or_imprecise_dtypes=True)

            dinv_my = cpool.tile([128, TT], F32)
            nc.sync.dma_start(dinv_my[:], dmy_in.ap())
            dinv2_my = cpool.tile([128, TT], F32)
            nc.sync.dma_start(dinv2_my[:], d2my_in.ap())
            sval_sb = cpool.tile([128, S // 128], BF16)
            nc.sync.dma_start(sval_sb[:], sval_in.ap())
            x_my = cpool.tile([128, TT * 8], F32)
            nc.sync.dma_start(x_my[:], xmy_in.ap())
            P_sb = cpool.tile([128, TT * num_graphs], BF16)
            nc.sync.dma_start(P_sb[:], P_in.ap())

            Wt, btl, bntl = [], [], []
            for l in range(3):
                w = cpool.tile([128, Fos[l]], BF16, tag=f"W{l}")
                nc.sync.dma_start(w[:Fs[l], :], W_in[l].ap())
                Wt.append(w)
                b = cpool.tile([128, 1], F32, tag=f"b{l}")
                nc.sync.dma_start(b[:Fos[l], :], b_in[l].ap())
                btl.append(b)
                bn = cpool.tile([128, 1], F32, tag=f"bn{l}")
                nc.sync.dma_start(bn[:Fos[l], :], bn_in[l].ap())
                bntl.append(bn)
            W4_sb = cpool.tile([128, 128], F32, tag="W4")
            nc.sync.dma_start(W4_sb[:64, :], W4_in.ap())
            b4_sb = cpool.tile([128, 1], F32, tag="b4")
            nc.sync.dma_start(b4_sb[:], b4_in.ap())
            Wlin_sb = cpool.tile([128, n_classes], F32, tag="wlin")
            nc.sync.dma_start(Wlin_sb[:], Wlin_in.ap())
            blin_sb = cpool.tile([num_graphs, n_classes], F32, tag="blin")
            nc.sync.dma_start(blin_sb[:], blin_in.ap())

            # ---------------- g1 = dinv * x (full, every core), bf16 packed
            ncols = Npad // 128
            CCH = 128
            xv = x_in.ap().rearrange("(p c) f -> p c f", p=128)
            g1v = g1.ap().rearrange("(p c) f -> p c f", p=128)
            for c0 in range(0, ncols, CCH):
                xt = mpool.tile([128, CCH, 8], BF16, tag="xt")
                nc.sync.dma_start(xt[:], xv[:, c0:c0 + CCH, :])
                da = mpool.tile([128, CCH], BF16, tag="da")
                nc.sync.dma_start(da[:], dall_in.ap()[:, c0:c0 + CCH])
                gt = mpool.tile([128, CCH, 8], BF16, tag="gt")
                nc.vector.tensor_tensor(
                    gt[:], xt[:],
                    da[:, :, None].broadcast_to([128, CCH, 8]),
                    op=mybir.AluOpType.mult)
                nc.sync.dma_start(g1v[:, c0:c0 + CCH, :], gt[:])

            gq = [0]
            aggT = bpool.tile([128, nodes_my // 2], BF16, tag="aggT")
            h_sbT = bpool.tile([128, nodes_my // 2], BF16, tag="h_sbT")
            agg = bpool.tile([128, TT * 32], BF16, tag="agg")
            gnext0 = bpool.tile([128, TT * 16], BF16, tag="gnext0")
            gnext1 = bpool.tile([128, TT * 32], BF16, tag="gnext1")
            h3buf = bpool.tile([128, TT * 64], BF16, tag="h3buf")
            IH = max(SH0, S - SH0) // 16
            idx_h = [bpool.tile([128, IH], I16, tag=f"idx_h{h}",
                                name=f"idx_h{h}") for h in range(2)]
            sub_d = [bpool.tile([128, S // 128], BF16, tag=f"sub_d{h}",
                                name=f"sub_d{h}") for h in range(2)]
            nc.sync.dma_start(idx_h[0][:, :SH0 // 16],
                              idx_in[0].ap()[:, :SH0 // 16])
            nc.sync.dma_start(idx_h[1][:, :(S - SH0) // 16],
                              idx_in[0].ap()[:, SH0 // 16:])
            nc.sync.dma_start(sub_d[0][:], sub_in[0].ap())
            pp_sb = cpool.tile([128, num_graphs], F32, tag="pp_sb")
            nc.vector.memset(pp_sb[:], 0.0)

            for li in range(3):
                F, Fo, k = Fs[li], Fos[li], ks[li]
                gprev = [None, gnext0, gnext1][li]
                gout = [gnext0, gnext1, None][li]
                sub_sb = sub_d[li % 2]
                gsrc = gs[li].ap().rearrange("(r k) f -> r (k f)", k=k)

                # ---- aggregation + post-phase, interleaved per node-half.
                # Flat 1024-idx gather stream; groups map to dst tiles via
                # position (tile t spans [t*L, (t+1)*L), L % 128 == 0).
                a_f = alphas[li]
                HT = TT // 2
                psums = {}

                def finish_group(g0, k=k, F=F, li=li, gprev=gprev):
                    # drain one 4-tile psum bank, slice-reduce each tile's k
                    # F-wide blocks (3D halving view), apply dinv scale and
                    # the self-loop term, write agg for 4 tiles at once.
                    t0 = 4 * g0
                    psum = psums.pop(g0)
                    red0 = rpool.tile([128, 512], F32, tag="red0")
                    nc.scalar.copy(red0[:], psum[:, :512])
                    cur = red0[:].rearrange("p (t m) -> p t m", t=4)
                    m = k * F
                    while m > F:
                        half = m // 2
                        nxt = rpool.tile([128, 4, 64], F32, tag=f"redt{m}")
                        nc.vector.tensor_tensor(
                            nxt[:, :, :half], cur[:, :, :half],
                            cur[:, :, half:m], op=mybir.AluOpType.add)
                        cur = nxt[:, :, :half]
                        m = half
                    sc4 = rpool.tile([128, 4, 32], F32, tag="sc4")
                    if li == 0:
                        nc.vector.tensor_tensor(
                            sc4[:, :, :F],
                            x_my[:, t0 * F:(t0 + 4) * F].rearrange(
                                "p (t f) -> p t f", f=F),
                            dinv2_my[:, t0:t0 + 4, None].broadcast_to(
                                [128, 4, F]),
                            op=mybir.AluOpType.mult)
                    else:
                        # gprev holds dinv*h, so one more dinv = dinv^2*h
                        nc.vector.tensor_tensor(
                            sc4[:, :, :F],
                            gprev[:, t0 * F:(t0 + 4) * F].rearrange(
                                "p (t f) -> p t f", f=F),
                            dinv_my[:, t0:t0 + 4, None].broadcast_to(
                                [128, 4, F]),
                            op=mybir.AluOpType.mult)
                    t24 = rpool.tile([128, 4, 32], F32, tag="t24")
                    nc.vector.tensor_tensor(
                        t24[:, :, :F], cur[:, :, :F],
                        dinv_my[:, t0:t0 + 4, None].broadcast_to([128, 4, F]),
                        op=mybir.AluOpType.mult)
                    nc.vector.tensor_tensor(
                        agg[:, t0 * F:(t0 + 4) * F].rearrange(
                            "p (t f) -> p t f", f=F),
                        t24[:, :, :F], sc4[:, :, :F],
                        op=mybir.AluOpType.add)

                cur_t = [0]
                for hb in range(2):
                  if True:
                    idx_sb = idx_h[hb]
                    hb0 = 0 if hb == 0 else SH0
                    hb1 = SH0 if hb == 0 else S
                    for base in range(hb0, hb1, GIDX):
                        nidx = GIDX
                        ng = nidx // 128
                        gt = gpool.tile([128, 8, 128], BF16, tag="gtile")
                        nc.gpsimd.dma_gather(
                            gt[:, :ng, :], gsrc,
                            idx_sb[:, (base - hb0) // 16:
                                   (base - hb0 + nidx) // 16],
                            nidx, nidx, 128, queue_num=gq[0] % 4)
                        gq[0] += 1
                        sv = sval_sb[:, base // 128:(base + nidx) // 128]
                        sb = sub_sb[:, base // 128:(base + nidx) // 128]
                        A = apool.tile([128, 8, 128], BF16, tag="A")
                        nc.vector.tensor_tensor(
                            A[:, :ng, :],
                            sv[:, :, None].broadcast_to([128, ng, 128]),
                            iota[:, None, :].broadcast_to([128, ng, 128]),
                            op=mybir.AluOpType.is_equal)
                        Mt = apool.tile([128, 128], BF16, tag="M")
                        nc.vector.tensor_tensor(
                            Mt[:, :ng * k].rearrange("p (g k) -> p g k", k=k),
                            sb[:, :, None].broadcast_to([128, ng, k]),
                            iota[:, None, :k].broadcast_to([128, ng, k]),
                            op=mybir.AluOpType.is_equal)
                        Gm = gpool.tile([128, 8, 128], BF16, tag="Gm")
                        nc.vector.tensor_tensor(
                            Gm[:, :ng, :].rearrange("p g (k f) -> p (g k) f",
                                                    k=k),
                            gt[:, :ng, :].rearrange("p g (k f) -> p (g k) f",
                                                    k=k),
                            Mt[:, :ng * k, None].broadcast_to([128, ng * k, F]),
                            op=mybir.AluOpType.mult)
                        for gg in range(ng):
                            pos = base + gg * 128
                            while pos >= prefix[cur_t[0] + 1]:
                                cur_t[0] += 1
                            t = cur_t[0]
                            g4 = t // 4
                            if g4 not in psums:
                                psums[g4] = pspool.tile(
                                    [128, 512], F32, tag=f"ps{g4 % 6}",
                                    name=f"pst{g4 % 6}")
                            c4 = (t % 4) * 128
                            nc.tensor.matmul(
                                psums[g4][:, c4:c4 + 128],
                                A[:, gg, :], Gm[:, gg, :],
                                start=(pos == prefix[t]),
                                stop=(pos == prefix[t + 1] - 128),
                                skip_group_check=True)
                            if pos == prefix[4 * g4 + 4] - 128:
                                finish_group(g4)
                    if li < 2:
                        nc.sync.dma_start(
                            idx_h[hb][:, :(hb1 - hb0) // 16],
                            idx_in[li + 1].ap()[:, hb0 // 16:hb1 // 16])
                        if hb == 0:
                            nc.sync.dma_start(sub_d[(li + 1) % 2][:],
                                              sub_in[li + 1].ap())

                  # ---- this half: transpose agg -> aggT, W matmul+PReLU,
                  #      transpose back, keep h_res, emit gnext / P-psum
                  if True:
                    t0 = hb * HT
                    for tt in range(HT):
                        t = t0 + tt
                        tp = pspool.tile([128, 512], BF16, tag=f"ps{6 + tt % 2}")
                        nc.tensor.matmul(tp[:F, :128],
                                         agg[:, t * F:(t + 1) * F],
                                         iden[:], is_transpose=True,
                                         skip_group_check=True)
                        nc.scalar.copy(aggT[:F, tt * 128:(tt + 1) * 128],
                                       tp[:F, :128])
                    for n0 in range(0, nodes_my // 2, 512):
                        hp = pspool.tile([128, 512], F32,
                                         tag=f"ps{6 + (n0 // 512) % 2}")
                        nc.tensor.matmul(hp[:Fo, :512], Wt[li][:F, :Fo],
                                         aggT[:F, n0:n0 + 512],
                                         skip_group_check=True)
                        # prelu(x+b) = relu(x+b) - a * relu(-x-b)
                        nc.scalar.activation(
                            h_sbT[:Fo, n0:n0 + 512], hp[:Fo, :512],
                            mybir.ActivationFunctionType.Relu,
                            bias=btl[li][:Fo, :], scale=1.0)
                        hrelu = mpool.tile([128, 512], BF16, tag="hrelu")
                        nc.scalar.activation(
                            hrelu[:Fo, :512], hp[:Fo, :512],
                            mybir.ActivationFunctionType.Relu,
                            bias=bntl[li][:Fo, :], scale=-1.0)
                        nc.vector.scalar_tensor_tensor(
                            h_sbT[:Fo, n0:n0 + 512], hrelu[:Fo, :512],
                            float(-a_f), h_sbT[:Fo, n0:n0 + 512],
                            op0=mybir.AluOpType.mult, op1=mybir.AluOpType.add)
                    for tt in range(HT):
                        t = t0 + tt
                        tb = pspool.tile([128, 512], BF16, tag=f"ps{6 + tt % 2}")
                        nc.tensor.matmul(tb[:128, :Fo],
                                         h_sbT[:Fo, tt * 128:(tt + 1) * 128],
                                         iden[:Fo, :Fo], is_transpose=True,
                                         skip_group_check=True)
                        if li < 2:
                            nc.scalar.activation(
                                gout[:, t * Fo:(t + 1) * Fo], tb[:, :Fo],
                                mybir.ActivationFunctionType.Identity,
                                scale=dinv_my[:, t:t + 1])
                        else:
                            nc.vector.tensor_copy(
                                h3buf[:, t * Fo:(t + 1) * Fo], tb[:, :Fo])
                            ppp = pspool.tile([128, 512], F32,
                                              tag=f"ps{6 + (tt + 1) % 2}")
                            nc.tensor.matmul(
                                ppp[:64, :num_graphs],
                                h3buf[:, t * Fo:(t + 1) * Fo],
                                P_sb[:, t * num_graphs:(t + 1) * num_graphs],
                                start=True, stop=True,
                                skip_group_check=True)
                            nc.vector.tensor_tensor(
                                pp_sb[:64, :], pp_sb[:64, :],
                                ppp[:64, :num_graphs],
                                op=mybir.AluOpType.add)

                    if li < 2:
                        hsv = hss[li].ap().rearrange("(p t) f -> p t f", p=128)
                        nc.sync.dma_start(
                            hsv[:, t0:t0 + HT, :],
                            gout[:, t0 * Fo:(t0 + HT) * Fo].rearrange(
                                "p (t f) -> p t f", f=Fo))
                if li < 2:
                    if n_cores > 1:
                        nc.gpsimd.collective_compute(
                            "AllGather", mybir.AluOpType.bypass, rg,
                            [hss[li].ap()], [gs[li + 1].ap()])
                    else:
                        nc.sync.dma_start(gs[li + 1].ap()[:nodes_my, :],
                                          hss[li].ap())

            # ---------------- tail: pooledT = W4^T @ ppT + b4; AllReduce; lin
            poolp = pspool.tile([128, 512], F32, tag="ps7")
            nc.tensor.matmul(poolp[:128, :num_graphs], W4_sb[:64, :128],
                             pp_sb[:64, :num_graphs], skip_group_check=True)
            poolT_sb = cpool.tile([128, num_graphs], F32, tag="poolT")
            nc.scalar.activation(poolT_sb[:], poolp[:128, :num_graphs],
                                 mybir.ActivationFunctionType.Identity,
                                 bias=b4_sb[:, :], scale=1.0)
            if n_cores > 1:
                nc.sync.dma_start(pooled_d.ap(), poolT_sb[:])
                nc.gpsimd.collective_compute(
                    "AllReduce", mybir.AluOpType.add, rg,
                    [pooled_d.ap()], [pooled_r.ap()])
                poolT2 = cpool.tile([128, num_graphs], F32, tag="poolT2")
                nc.sync.dma_start(poolT2[:], pooled_r.ap())
            else:
                poolT2 = poolT_sb
            fin = pspool.tile([128, 512], F32, tag="ps6")
            nc.tensor.matmul(fin[:num_graphs, :n_classes], poolT2[:],
                             Wlin_sb[:], skip_group_check=True)
            out_sb = cpool.tile([num_graphs, n_classes], F32, tag="outsb")
            nc.vector.tensor_tensor(out_sb[:], fin[:num_graphs, :n_classes],
                                    blin_sb[:], op=mybir.AluOpType.add)
            nc.sync.dma_start(out_t.ap(), out_sb[:])

    nc.compile()
    return nc


# ------------------------------------------------------------------ entry
def kernel(x, edge_src, edge_dst, batch,
           W1, b1, W2, b2, W3, b3, W4, b4,
           a1, a2, a3, Wlin, blin, n_cores=NCORES):
    x = np.asarray(x, dtype=np.float32)
    edge_src = np.asarray(edge_src, dtype=np.int32)
    edge_dst = np.asarray(edge_dst, dtype=np.int32)
    batch = np.asarray(batch, dtype=np.int32)
    Ws = [np.asarray(w, np.float32) for w in (W1, W2, W3, W4)]
    bs = [np.asarray(b, np.float32) for b in (b1, b2, b3, b4)]
    alphas = [float(a1), float(a2), float(a3)]
    Wlin = np.asarray(Wlin, np.float32)
    blin = np.asarray(blin, np.float32)
    NG, NCLS = 64, Wlin.shape[1]

    meta = _preprocess(x, edge_src, edge_dst, batch, NG)
    nc = _build(meta, NG, NCLS, alphas, n_cores)
    in_maps = _in_maps(meta, Ws, bs, Wlin, blin, NG, n_cores)
    res = run_bass_kernel_spmd(nc, in_maps, core_ids=list(range(n_cores)))
    return np.asarray(res.results[0]["out"], dtype=np.float32)


def _in_maps(meta, Ws, bs, Wlin, blin, NG, n_cores=NCORES):
    in_maps = []
    for c in range(n_cores):
        m = dict(
            x_perm=meta["x_perm"],
            x_my=np.ascontiguousarray(meta["x_my"][c]),
            dinv_all=meta["dinv_all"],
            dinv_my=np.ascontiguousarray(meta["dinv_my"][c]),
            dinv2_my=np.ascontiguousarray(meta["dinv2_my"][c]),
            sval=np.asarray(meta["sval_tbl"][c]),
            P_my=np.asarray(meta["P_my"][c]),
            W4=Ws[3].astype(np.float32),
            b4=np.ascontiguousarray(bs[3].reshape(-1, 1)),
            Wlin=Wlin,
            blin_rep=np.tile(blin[None, :], (NG, 1)).astype(np.float32),
        )
        for l in range(3):
            m[f"idx{l+1}"] = np.asarray(meta["idx_tbl"][l, c])
            m[f"sub{l+1}"] = np.asarray(meta["sub_tbl"][l][c])
            m[f"W{l+1}"] = Ws[l].astype(ml_dtypes.bfloat16)
            m[f"b{l+1}"] = np.ascontiguousarray(bs[l].reshape(-1, 1))
            m[f"bn{l+1}"] = np.ascontiguousarray(-bs[l].reshape(-1, 1))
        in_maps.append(m)
    return in_maps


# revision 33
# speedup vs baseline: 1.0889x; 1.0413x over previous
"""GCN forward (4-layer GCNConv + global mean-pool + linear) on 8 TRN2 cores.

Redesign vs. the v1 kernel (dst-tile dma_gather at 256B/edge, 4 layers):
  * Layer 4 + mean-pool are collapsed into a host-built structure matrix
    P[graph, node] (= pool(1/cnt) . A_hat norms): pooled = (P @ h3) @ W4 + b4.
    The widest aggregation (F=64) becomes a tiny dense TensorE contraction.
  * Gather rows are packed: h stored contiguously [Npad, F] bf16 so one 256B
    dma_gather row holds k = 128/F nodes (16/8/4 for F=8/16/32). An int16 row
    index then covers all of Npad -> no src chunking, and per-(tile) padding
    only (to 128) -> ~213k descriptors/layer/core vs 416k before.
  * Per-edge sub-row selection is a DVE mask (sub == iota_k) multiplied into
    the gathered rows; the one-hot scatter matmul then accumulates a full
    [128 dst, 128col] psum whose k F-wide column blocks are slice-reduced.
  * Self-loop term is computed directly as dinv^2 * h per dst tile (DVE),
    never gathered.
  * Aggregation math: agg_d = dinv_d * sum_{s->d} (dinv_s h_s) + dinv_d^2 h_d,
    with dinv_s folded into the stored gather source.

All graph preprocessing (tile packing, index/mask tables, P) is host numpy
and depends only on graph structure (edges/batch/degrees), never on x or W.
"""

import heapq

import numpy as np
import ml_dtypes

import concourse.bacc as bacc
import concourse.mybir as mybir
import concourse.tile as tile
from concourse.bass_utils import run_bass_kernel_spmd
from concourse.library_config import mlp as mlp_lib
from concourse.masks import make_identity

F32 = mybir.dt.float32
BF16 = mybir.dt.bfloat16
I16 = mybir.dt.int16
FP8 = mybir.dt.float8e4

NCORES = 8
TT = 128              # dst tiles per core
NPC = TT * 128        # dst nodes per core (16384)
GIDX = 1024           # max idxs per dma_gather instruction (SWDGE ring limit)
PAD_S = 255.0         # slot/sub id for padding positions


# ------------------------------------------------------------------ host prep
def _preprocess(x, edge_src, edge_dst, batch, num_graphs):
    N = x.shape[0]
    Npad = NCORES * NPC
    indeg = np.bincount(edge_dst, minlength=N).astype(np.int64)
    deg = indeg + 1  # self loop
    dinv = (1.0 / np.sqrt(deg.astype(np.float64))).astype(np.float32)

    # Two-tier greedy assignment of dst nodes to (core, tile) bins by
    # indegree: tiles 0..123 target just under 12 gather groups (1536 slots),
    # the last 4 tiles per core absorb the excess -> ~5% fewer pad slots.
    order = np.argsort(-indeg, kind="stable")
    nbins = NCORES * TT
    NHI = 4
    cap_lo = 12 * 128 - 10.0
    t_hi = (len(edge_dst) / NCORES - (TT - NHI) * cap_lo) / NHI
    target = np.full(TT, cap_lo)
    for b_hi in (0, 1, TT // 2, TT // 2 + 1):
        target[b_hi] = max(t_hi, cap_lo)
    heap = [(-target[b % TT], b) for b in range(nbins)]
    heapq.heapify(heap)
    counts = np.zeros(nbins, np.int64)
    loads = np.zeros(nbins, np.float64)
    gid = np.empty(N, np.int64)
    for v in order:
        key, b = heapq.heappop(heap)
        c, t = b // TT, b % TT
        gid[v] = c * NPC + counts[b] * TT + t
        counts[b] += 1
        loads[b] += indeg[v]
        if counts[b] < 128:
            heapq.heappush(heap, (loads[b] - target[t], b))
    assert counts.max() <= 128

    dinv_pad = np.ones(Npad, np.float32)
    dinv_pad[gid] = dinv

    sg, dg = gid[edge_src], gid[edge_dst]
    core_e = dg // NPC
    tile_e = dg % TT
    slot_e = (dg % NPC) // TT
    binid = core_e * TT + tile_e
    cnts = np.bincount(binid, minlength=nbins).reshape(NCORES, TT)
    Ls = (-(-cnts.max(axis=0) // 128) * 128).astype(np.int64)
    Ls[TT // 2 - 1] += (-Ls[:TT // 2].sum()) % 1024     # align half streams
    Ls[TT - 1] += (-Ls[TT // 2:].sum()) % 1024
    prefix = np.concatenate([[0], np.cumsum(Ls)])
    S = int(prefix[-1])

    ks = (16, 8, 4)
    order_e = np.argsort(binid, kind="stable")
    sortedbin = binid[order_e]
    run_start = np.searchsorted(sortedbin, np.arange(nbins))
    rank = np.arange(len(order_e)) - run_start[sortedbin]
    pos = prefix[sortedbin % TT] + rank
    core_arr = sortedbin // TT

    idx_flat = np.zeros((3, NCORES, S), np.int16)
    sub_flat = np.full((3, NCORES, S), PAD_S, np.float32)
    sval_flat = np.full((NCORES, S), PAD_S, np.float32)
    for c in range(NCORES):
        m = core_arr == c
        es, p = order_e[m], pos[m]
        sval_flat[c, p] = slot_e[es].astype(np.float32)
        for li, k in enumerate(ks):
            idx_flat[li, c, p] = (sg[es] // k).astype(np.int16)
            sub_flat[li, c, p] = (sg[es] % k).astype(np.float32)

    def wrap16(a):  # [S] -> [128, S//16]
        return np.tile(a.reshape(S // 16, 16).T, (8, 1)).copy()

    def wrap128(a):  # [S] -> [128, S//128]
        return a.reshape(S // 128, 128).T.astype(ml_dtypes.bfloat16)

    idx_tbl = np.stack([[wrap16(idx_flat[li, c]) for c in range(NCORES)]
                        for li in range(3)])
    sub_tbl = np.stack([[wrap128(sub_flat[li, c]) for c in range(NCORES)]
                        for li in range(3)])
    sval_tbl = np.stack([wrap128(sval_flat[c]) for c in range(NCORES)])

    x_perm = np.zeros((Npad, x.shape[1]), np.float32)
    x_perm[gid] = x
    dinv_all = dinv_pad.reshape(128, Npad // 128).astype(ml_dtypes.bfloat16)
    dinv_my = dinv_pad.reshape(NCORES, 128, TT).copy()
    dinv2_my = (dinv_my * dinv_my).copy()
    x_my = x_perm.reshape(NCORES, 128, TT * x.shape[1]).copy()    # [8,128,TT*8]

    cnt = np.bincount(batch, minlength=num_graphs).astype(np.float64)
    invc = (1.0 / np.maximum(cnt, 1.0)).astype(np.float64)
    bd = batch[edge_dst].astype(np.int64)
    w = dinv[edge_src].astype(np.float64) * dinv[edge_dst] * invc[bd]
    Pacc = np.bincount(sg * num_graphs + bd, weights=w,
                       minlength=Npad * num_graphs)
    Pacc += np.bincount(gid * num_graphs + batch.astype(np.int64),
                        weights=(dinv.astype(np.float64) ** 2) * invc[batch],
                        minlength=Npad * num_graphs)
    Pacc = Pacc.reshape(Npad, num_graphs).astype(np.float32)
    P_my = Pacc.reshape(NCORES, 128, TT * num_graphs).astype(ml_dtypes.bfloat16)

    x_perm = x_perm.astype(ml_dtypes.bfloat16)
    return dict(prefix=prefix, S=S, Npad=Npad, idx_tbl=idx_tbl, sub_tbl=sub_tbl,
                sval_tbl=sval_tbl, x_perm=x_perm, x_my=x_my,
                dinv_all=dinv_all, dinv_my=dinv_my, dinv2_my=dinv2_my,
                P_my=P_my)


# ------------------------------------------------------------------ device IR
def _build(meta, num_graphs, n_classes, alphas, n_cores=NCORES):
    S, Npad = meta["S"], meta["Npad"]
    prefix = [int(v) for v in meta["prefix"]]
    SH0 = prefix[TT // 2]
    Fs = [8, 16, 32]          # aggregation widths, layers 1-3
    Fos = [16, 32, 64]        # output widths, layers 1-3
    ks = [16, 8, 4]           # nodes per 256B gather row
    nodes_my = NPC

    nc = bacc.Bacc("TRN2", target_bir_lowering=False, debug=False,
                   num_devices=n_cores, num_swdge_queues=4)
    rg = [list(range(n_cores))]

    x_in = nc.dram_tensor("x_perm", [Npad, 8], BF16, kind="ExternalInput")
    xmy_in = nc.dram_tensor("x_my", [128, TT * 8], F32, kind="ExternalInput")
    dall_in = nc.dram_tensor("dinv_all", [128, Npad // 128], BF16,
                             kind="ExternalInput")
    dmy_in = nc.dram_tensor("dinv_my", [128, TT], F32, kind="ExternalInput")
    d2my_in = nc.dram_tensor("dinv2_my", [128, TT], F32, kind="ExternalInput")
    idx_in = [nc.dram_tensor(f"idx{l+1}", [128, S // 16], I16,
                             kind="ExternalInput") for l in range(3)]
    sub_in = [nc.dram_tensor(f"sub{l+1}", [128, S // 128], BF16,
                             kind="ExternalInput") for l in range(3)]
    sval_in = nc.dram_tensor("sval", [128, S // 128], BF16, kind="ExternalInput")
    P_in = nc.dram_tensor("P_my", [128, TT * num_graphs], BF16,
                          kind="ExternalInput")
    W_in = [nc.dram_tensor(f"W{l+1}", [Fs[l], Fos[l]], BF16,
                           kind="ExternalInput") for l in range(3)]
    b_in = [nc.dram_tensor(f"b{l+1}", [Fos[l], 1], F32, kind="ExternalInput")
            for l in range(3)]
    bn_in = [nc.dram_tensor(f"bn{l+1}", [Fos[l], 1], F32, kind="ExternalInput")
             for l in range(3)]
    W4_in = nc.dram_tensor("W4", [64, 128], F32, kind="ExternalInput")
    b4_in = nc.dram_tensor("b4", [128, 1], F32, kind="ExternalInput")
    Wlin_in = nc.dram_tensor("Wlin", [128, n_classes], F32, kind="ExternalInput")
    blin_in = nc.dram_tensor("blin_rep", [num_graphs, n_classes], F32,
                             kind="ExternalInput")
    out_t = nc.dram_tensor("out", [num_graphs, n_classes], F32,
                           kind="ExternalOutput")

    g1 = nc.dram_tensor("g1", [Npad, 8], BF16)
    g2 = nc.dram_tensor("g2", [Npad, 16], BF16, addr_space="Shared")
    g3 = nc.dram_tensor("g3", [Npad, 32], BF16, addr_space="Shared")
    gs = [g1, g2, g3]
    hs2 = nc.dram_tensor("hs2", [nodes_my, 16], BF16)
    hs3 = nc.dram_tensor("hs3", [nodes_my, 32], BF16)
    hss = [hs2, hs3]
    pooled_d = nc.dram_tensor("pooled", [128, num_graphs], F32)
    pooled_r = nc.dram_tensor("pooled_red", [128, num_graphs], F32, addr_space="Shared")

    with tile.TileContext(nc) as tc:
        with (
            tc.tile_pool(name="const", bufs=1) as cpool,
            tc.tile_pool(name="meta", bufs=2) as mpool,
            tc.tile_pool(name="gat", bufs=8) as gpool,
            tc.tile_pool(name="am", bufs=8) as apool,
            tc.tile_pool(name="red", bufs=4) as rpool,
            tc.tile_pool(name="big", bufs=1) as bpool,
            tc.tile_pool(name="ps", bufs=1, space="PSUM") as pspool,
        ):
            nc.gpsimd.load_library(mlp_lib)

            iden = cpool.tile([128, 128], BF16)
            make_identity(nc, iden[:])
            iota = cpool.tile([128, 128], BF16)
            nc.gpsimd.iota(iota[:], [[1, 128]], channel_multiplier=0,
                           allow_small_or_imprecise_dtypes=True)

            dinv_my = cpool.tile([128, TT], F32)
            nc.sync.dma_start(dinv_my[:], dmy_in.ap())
            dinv2_my = cpool.tile([128, TT], F32)
            nc.sync.dma_start(dinv2_my[:], d2my_in.ap())
            sval_sb = cpool.tile([128, S // 128], BF16)
            nc.sync.dma_start(sval_sb[:], sval_in.ap())
            x_my = cpool.tile([128, TT * 8], F32)
            nc.sync.dma_start(x_my[:], xmy_in.ap())
            P_sb = cpool.tile([128, TT * num_graphs], BF16)
            nc.sync.dma_start(P_sb[:], P_in.ap())

            Wt, btl, bntl = [], [], []
            for l in range(3):
                w = cpool.tile([128, Fos[l]], BF16, tag=f"W{l}")
                nc.sync.dma_start(w[:Fs[l], :], W_in[l].ap())
                Wt.append(w)
                b = cpool.tile([128, 1], F32, tag=f"b{l}")
                nc.sync.dma_start(b[:Fos[l], :], b_in[l].ap())
                btl.append(b)
                bn = cpool.tile([128, 1], F32, tag=f"bn{l}")
                nc.sync.dma_start(bn[:Fos[l], :], bn_in[l].ap())
                bntl.append(bn)
            W4_sb = cpool.tile([128, 128], F32, tag="W4")
            nc.sync.dma_start(W4_sb[:64, :], W4_in.ap())
            b4_sb = cpool.tile([128, 1], F32, tag="b4")
            nc.sync.dma_start(b4_sb[:], b4_in.ap())
            Wlin_sb = cpool.tile([128, n_classes], F32, tag="wlin")
            nc.sync.dma_start(Wlin_sb[:], Wlin_in.ap())
            blin_sb = cpool.tile([num_graphs, n_classes], F32, tag="blin")
            nc.sync.dma_start(blin_sb[:], blin_in.ap())

            # ---------------- g1 = dinv * x (full, every core), bf16 packed
            ncols = Npad // 128
            CCH = 128
            xv = x_in.ap().rearrange("(p c) f -> p c f", p=128)
            g1v = g1.ap().rearrange("(p c) f -> p c f", p=128)
            for c0 in range(0, ncols, CCH):
                xt = mpool.tile([128, CCH, 8], BF16, tag="xt")
                nc.sync.dma_start(xt[:], xv[:, c0:c0 + CCH, :])
                da = mpool.tile([128, CCH], BF16, tag="da")
                nc.sync.dma_start(da[:], dall_in.ap()[:, c0:c0 + CCH])
                gt = mpool.tile([128, CCH, 8], BF16, tag="gt")
                nc.vector.tensor_tensor(
                    gt[:], xt[:],
                    da[:, :, None].broadcast_to([128, CCH, 8]),
                    op=mybir.AluOpType.mult)
                nc.sync.dma_start(g1v[:, c0:c0 + CCH, :], gt[:])

            gq = [0]
            aggT = bpool.tile([128, nodes_my // 2], BF16, tag="aggT")
            h_sbT = bpool.tile([128, nodes_my // 2], BF16, tag="h_sbT")
            agg = bpool.tile([128, TT * 32], BF16, tag="agg")
            gnext0 = bpool.tile([128, TT * 16], BF16, tag="gnext0")
            gnext1 = bpool.tile([128, TT * 32], BF16, tag="gnext1")
            h3buf = bpool.tile([128, TT * 64], BF16, tag="h3buf")
            IH = max(SH0, S - SH0) // 16
            idx_h = [bpool.tile([128, IH], I16, tag=f"idx_h{h}",
                                name=f"idx_h{h}") for h in range(2)]
            sub_d = [bpool.tile([128, S // 128], BF16, tag=f"sub_d{h}",
                                name=f"sub_d{h}") for h in range(2)]
            nc.sync.dma_start(idx_h[0][:, :SH0 // 16],
                              idx_in[0].ap()[:, :SH0 // 16])
            nc.sync.dma_start(idx_h[1][:, :(S - SH0) // 16],
                              idx_in[0].ap()[:, SH0 // 16:])
            nc.sync.dma_start(sub_d[0][:], sub_in[0].ap())
            pp_sb = cpool.tile([128, num_graphs], F32, tag="pp_sb")
            nc.vector.memset(pp_sb[:], 0.0)

            for li in range(3):
                F, Fo, k = Fs[li], Fos[li], ks[li]
                gprev = [None, gnext0, gnext1][li]
                gout = [gnext0, gnext1, None][li]
                sub_sb = sub_d[li % 2]
                gsrc = gs[li].ap().rearrange("(r k) f -> r (k f)", k=k)

                # ---- aggregation + post-phase, interleaved per node-half.
                # Flat 1024-idx gather stream; groups map to dst tiles via
                # position (tile t spans [t*L, (t+1)*L), L % 128 == 0).
                a_f = alphas[li]
                HT = TT // 2
                psums = {}

                def finish_group(g0, k=k, F=F, li=li, gprev=gprev):
                    # drain one 4-tile psum bank, slice-reduce each tile's k
                    # F-wide blocks (3D halving view), apply dinv scale and
                    # the self-loop term, write agg for 4 tiles at once.
                    t0 = 4 * g0
                    psum = psums.pop(g0)
                    red0 = rpool.tile([128, 512], F32, tag="red0")
                    nc.scalar.copy(red0[:], psum[:, :512])
                    cur = red0[:].rearrange("p (t m) -> p t m", t=4)
                    m = k * F
                    while m > F:
                        half = m // 2
                        nxt = rpool.tile([128, 4, 64], F32, tag=f"redt{m}")
                        nc.vector.tensor_tensor(
                            nxt[:, :, :half], cur[:, :, :half],
                            cur[:, :, half:m], op=mybir.AluOpType.add)
                        cur = nxt[:, :, :half]
                        m = half
                    sc4 = rpool.tile([128, 4, 32], F32, tag="sc4")
                    if li == 0:
                        nc.vector.tensor_tensor(
                            sc4[:, :, :F],
                            x_my[:, t0 * F:(t0 + 4) * F].rearrange(
                                "p (t f) -> p t f", f=F),
                            dinv2_my[:, t0:t0 + 4, None].broadcast_to(
                                [128, 4, F]),
                            op=mybir.AluOpType.mult)
                    else:
                        # gprev holds dinv*h, so one more dinv = dinv^2*h
                        nc.vector.tensor_tensor(
                            sc4[:, :, :F],
                            gprev[:, t0 * F:(t0 + 4) * F].rearrange(
                                "p (t f) -> p t f", f=F),
                            dinv_my[:, t0:t0 + 4, None].broadcast_to(
                                [128, 4, F]),
                            op=mybir.AluOpType.mult)
                    t24 = rpool.tile([128, 4, 32], F32, tag="t24")
                    nc.vector.tensor_tensor(
                        t24[:, :, :F], cur[:, :, :F],
                        dinv_my[:, t0:t0 + 4, None].broadcast_to([128, 4, F]),
                        op=mybir.AluOpType.mult)
                    nc.vector.tensor_tensor(
                        agg[:, t0 * F:(t0 + 4) * F].rearrange(
                            "p (t f) -> p t f", f=F),
                        t24[:, :, :F], sc4[:, :, :F],
                        op=mybir.AluOpType.add)

                cur_t = [0]
                for hb in range(2):
                  if True:
                    idx_sb = idx_h[hb]
                    hb0 = 0 if hb == 0 else SH0
                    hb1 = SH0 if hb == 0 else S
                    for base in range(hb0, hb1, GIDX):
                        nidx = GIDX
                        ng = nidx // 128
                        gt = gpool.tile([128, 8, 128], BF16, tag="gtile")
                        nc.gpsimd.dma_gather(
                            gt[:, :ng, :], gsrc,
                            idx_sb[:, (base - hb0) // 16:
                                   (base - hb0 + nidx) // 16],
                            nidx, nidx, 128, queue_num=gq[0] % 4)
                        gq[0] += 1
                        sv = sval_sb[:, base // 128:(base + nidx) // 128]
                        sb = sub_sb[:, base // 128:(base + nidx) // 128]
                        A = apool.tile([128, 8, 128], FP8, tag="A")
                        nc.vector.tensor_tensor(
                            A[:, :ng, :],
                            sv[:, :, None].broadcast_to([128, ng, 128]),
                            iota[:, None, :].broadcast_to([128, ng, 128]),
                            op=mybir.AluOpType.is_equal)
                        Mt = apool.tile([128, 128], BF16, tag="M")
                        nc.vector.tensor_tensor(
                            Mt[:, :ng * k].rearrange("p (g k) -> p g k", k=k),
                            sb[:, :, None].broadcast_to([128, ng, k]),
                            iota[:, None, :k].broadcast_to([128, ng, k]),
                            op=mybir.AluOpType.is_equal)
                        Gm = gpool.tile([128, 8, 128], FP8, tag="Gm")
                        nc.vector.tensor_tensor(
                            Gm[:, :ng, :].rearrange("p g (k f) -> p (g k) f",
                                                    k=k),
                            gt[:, :ng, :].rearrange("p g (k f) -> p (g k) f",
                                                    k=k),
                            Mt[:, :ng * k, None].broadcast_to([128, ng * k, F]),
                            op=mybir.AluOpType.mult)
                        for gg in range(ng):
                            pos = base + gg * 128
                            while pos >= prefix[cur_t[0] + 1]:
                                cur_t[0] += 1
                            t = cur_t[0]
                            g4 = t // 4
                            if g4 not in psums:
                                psums[g4] = pspool.tile(
                                    [128, 512], F32, tag=f"ps{g4 % 6}",
                                    name=f"pst{g4 % 6}")
                            c4 = (t % 4) * 128
                            nc.tensor.matmul(
                                psums[g4][:, c4:c4 + 128],
                                A[:, gg, :], Gm[:, gg, :],
                                start=(pos == prefix[t]),
                                stop=(pos == prefix[t + 1] - 128),
                                skip_group_check=True)
                            if pos == prefix[4 * g4 + 4] - 128:
                                finish_group(g4)
                    if li < 2:
                        nc.sync.dma_start(
                            idx_h[hb][:, :(hb1 - hb0) // 16],
                            idx_in[li + 1].ap()[:, hb0 // 16:hb1 // 16])
                        if hb == 0:
                            nc.sync.dma_start(sub_d[(li + 1) % 2][:],
                                              sub_in[li + 1].ap())

                  # ---- this half: transpose agg -> aggT, W matmul+PReLU,
                  #      transpose back, keep h_res, emit gnext / P-psum
                  if True:
                    t0 = hb * HT
                    for tt in range(HT):
                        t = t0 + tt
                        tp = pspool.tile([128, 512], BF16, tag=f"ps{6 + tt % 2}")
                        nc.tensor.matmul(tp[:F, :128],
                                         agg[:, t * F:(t + 1) * F],
                                         iden[:], is_transpose=True,
                                         skip_group_check=True)
                        nc.scalar.copy(aggT[:F, tt * 128:(tt + 1) * 128],
                                       tp[:F, :128])
                    for n0 in range(0, nodes_my // 2, 512):
                        hp = pspool.tile([128, 512], F32,
                                         tag=f"ps{6 + (n0 // 512) % 2}")
                        nc.tensor.matmul(hp[:Fo, :512], Wt[li][:F, :Fo],
                                         aggT[:F, n0:n0 + 512],
                                         skip_group_check=True)
                        # prelu(x+b) = relu(x+b) - a * relu(-x-b)
                        nc.scalar.activation(
                            h_sbT[:Fo, n0:n0 + 512], hp[:Fo, :512],
                            mybir.ActivationFunctionType.Relu,
                            bias=btl[li][:Fo, :], scale=1.0)
                        hrelu = mpool.tile([128, 512], BF16, tag="hrelu")
                        nc.scalar.activation(
                            hrelu[:Fo, :512], hp[:Fo, :512],
                            mybir.ActivationFunctionType.Relu,
                            bias=bntl[li][:Fo, :], scale=-1.0)
                        nc.vector.scalar_tensor_tensor(
                            h_sbT[:Fo, n0:n0 + 512], hrelu[:Fo, :512],
                            float(-a_f), h_sbT[:Fo, n0:n0 + 512],
                            op0=mybir.AluOpType.mult, op1=mybir.AluOpType.add)
                    for tt in range(HT):
                        t = t0 + tt
                        tb = pspool.tile([128, 512], BF16, tag=f"ps{6 + tt % 2}")
                        nc.tensor.matmul(tb[:128, :Fo],
                                         h_sbT[:Fo, tt * 128:(tt + 1) * 128],
                                         iden[:Fo, :Fo], is_transpose=True,
                                         skip_group_check=True)
                        if li < 2:
                            nc.scalar.activation(
                                gout[:, t * Fo:(t + 1) * Fo], tb[:, :Fo],
                                mybir.ActivationFunctionType.Identity,
                                scale=dinv_my[:, t:t + 1])
                        else:
                            nc.vector.tensor_copy(
                                h3buf[:, t * Fo:(t + 1) * Fo], tb[:, :Fo])
                            ppp = pspool.tile([128, 512], F32,
                                              tag=f"ps{6 + (tt + 1) % 2}")
                            nc.tensor.matmul(
                                ppp[:64, :num_graphs],
                                h3buf[:, t * Fo:(t + 1) * Fo],
                                P_sb[:, t * num_graphs:(t + 1) * num_graphs],
                                start=True, stop=True,
                                skip_group_check=True)
                            nc.vector.tensor_tensor(
                                pp_sb[:64, :], pp_sb[:64, :],
                                ppp[:64, :num_graphs],
                                op=mybir.AluOpType.add)

                    if li < 2:
                        Foh = Fos[li]
                        th0 = hb * HT
                        hsv = hss[li].ap().rearrange("(p t) f -> p t f", p=128)
                        nc.sync.dma_start(
                            hsv[:, th0:th0 + HT, :],
                            gout[:, th0 * Foh:(th0 + HT) * Foh].rearrange(
                                "p (t f) -> p t f", f=Foh))
                if li < 2:
                    if n_cores > 1:
                        nc.gpsimd.collective_compute(
                            "AllGather", mybir.AluOpType.bypass, rg,
                            [hss[li].ap()], [gs[li + 1].ap()])
                    else:
                        nc.sync.dma_start(gs[li + 1].ap()[:nodes_my, :],
                                          hss[li].ap())

            # ---------------- tail: pooledT = W4^T @ ppT + b4; AllReduce; lin
            poolp = pspool.tile([128, 512], F32, tag="ps7")
            nc.tensor.matmul(poolp[:128, :num_graphs], W4_sb[:64, :128],
                             pp_sb[:64, :num_graphs], skip_group_check=True)
            poolT_sb = cpool.tile([128, num_graphs], F32, tag="poolT")
            nc.scalar.activation(poolT_sb[:], poolp[:128, :num_graphs],
                                 mybir.ActivationFunctionType.Identity,
                                 bias=b4_sb[:, :], scale=1.0)
            if n_cores > 1:
                nc.sync.dma_start(pooled_d.ap(), poolT_sb[:])
                nc.gpsimd.collective_compute(
                    "AllReduce", mybir.AluOpType.add, rg,
                    [pooled_d.ap()], [pooled_r.ap()])
                poolT2 = cpool.tile([128, num_graphs], F32, tag="poolT2")
                nc.sync.dma_start(poolT2[:], pooled_r.ap())
            else:
                poolT2 = poolT_sb
            fin = pspool.tile([128, 512], F32, tag="ps6")
            nc.tensor.matmul(fin[:num_graphs, :n_classes], poolT2[:],
                             Wlin_sb[:], skip_group_check=True)
            out_sb = cpool.tile([num_graphs, n_classes], F32, tag="outsb")
            nc.vector.tensor_tensor(out_sb[:], fin[:num_graphs, :n_classes],
                                    blin_sb[:], op=mybir.AluOpType.add)
            nc.sync.dma_start(out_t.ap(), out_sb[:])

    nc.compile()
    return nc


# ------------------------------------------------------------------ entry
def kernel(x, edge_src, edge_dst, batch,
           W1, b1, W2, b2, W3, b3, W4, b4,
           a1, a2, a3, Wlin, blin, n_cores=NCORES):
    x = np.asarray(x, dtype=np.float32)
    edge_src = np.asarray(edge_src, dtype=np.int32)
    edge_dst = np.asarray(edge_dst, dtype=np.int32)
    batch = np.asarray(batch, dtype=np.int32)
    Ws = [np.asarray(w, np.float32) for w in (W1, W2, W3, W4)]
    bs = [np.asarray(b, np.float32) for b in (b1, b2, b3, b4)]
    alphas = [float(a1), float(a2), float(a3)]
    Wlin = np.asarray(Wlin, np.float32)
    blin = np.asarray(blin, np.float32)
    NG, NCLS = 64, Wlin.shape[1]

    meta = _preprocess(x, edge_src, edge_dst, batch, NG)
    nc = _build(meta, NG, NCLS, alphas, n_cores)
    in_maps = _in_maps(meta, Ws, bs, Wlin, blin, NG, n_cores)
    res = run_bass_kernel_spmd(nc, in_maps, core_ids=list(range(n_cores)))
    return np.asarray(res.results[0]["out"], dtype=np.float32)


def _in_maps(meta, Ws, bs, Wlin, blin, NG, n_cores=NCORES):
    in_maps = []
    for c in range(n_cores):
        m = dict(
            x_perm=meta["x_perm"],
            x_my=np.ascontiguousarray(meta["x_my"][c]),
            dinv_all=meta["dinv_all"],
            dinv_my=np.ascontiguousarray(meta["dinv_my"][c]),
            dinv2_my=np.ascontiguousarray(meta["dinv2_my"][c]),
            sval=np.asarray(meta["sval_tbl"][c]),
            P_my=np.asarray(meta["P_my"][c]),
            W4=Ws[3].astype(np.float32),
            b4=np.ascontiguousarray(bs[3].reshape(-1, 1)),
            Wlin=Wlin,
            blin_rep=np.tile(blin[None, :], (NG, 1)).astype(np.float32),
        )
        for l in range(3):
            m[f"idx{l+1}"] = np.asarray(meta["idx_tbl"][l, c])
            m[f"sub{l+1}"] = np.asarray(meta["sub_tbl"][l][c])
            m[f"W{l+1}"] = Ws[l].astype(ml_dtypes.bfloat16)
            m[f"b{l+1}"] = np.ascontiguousarray(bs[l].reshape(-1, 1))
            m[f"bn{l+1}"] = np.ascontiguousarray(-bs[l].reshape(-1, 1))
        in_maps.append(m)
    return in_maps


# revision 35
# speedup vs baseline: 1.0939x; 1.0046x over previous
"""GCN forward (4-layer GCNConv + global mean-pool + linear) on 8 TRN2 cores.

Redesign vs. the v1 kernel (dst-tile dma_gather at 256B/edge, 4 layers):
  * Layer 4 + mean-pool are collapsed into a host-built structure matrix
    P[graph, node] (= pool(1/cnt) . A_hat norms): pooled = (P @ h3) @ W4 + b4.
    The widest aggregation (F=64) becomes a tiny dense TensorE contraction.
  * Gather rows are packed: h stored contiguously [Npad, F] bf16 so one 256B
    dma_gather row holds k = 128/F nodes (16/8/4 for F=8/16/32). An int16 row
    index then covers all of Npad -> no src chunking, and per-(tile) padding
    only (to 128) -> ~213k descriptors/layer/core vs 416k before.
  * Per-edge sub-row selection is a DVE mask (sub == iota_k) multiplied into
    the gathered rows; the one-hot scatter matmul then accumulates a full
    [128 dst, 128col] psum whose k F-wide column blocks are slice-reduced.
  * Self-loop term is computed directly as dinv^2 * h per dst tile (DVE),
    never gathered.
  * Aggregation math: agg_d = dinv_d * sum_{s->d} (dinv_s h_s) + dinv_d^2 h_d,
    with dinv_s folded into the stored gather source.

All graph preprocessing (tile packing, index/mask tables, P) is host numpy
and depends only on graph structure (edges/batch/degrees), never on x or W.
"""

import heapq

import numpy as np
import ml_dtypes

import concourse.bacc as bacc
import concourse.mybir as mybir
import concourse.tile as tile
from concourse.bass_utils import run_bass_kernel_spmd
from concourse.library_config import mlp as mlp_lib
from concourse.masks import make_identity

F32 = mybir.dt.float32
BF16 = mybir.dt.bfloat16
I16 = mybir.dt.int16
FP8 = mybir.dt.float8e4

NCORES = 8
TT = 128              # dst tiles per core
NPC = TT * 128        # dst nodes per core (16384)
GIDX = 1024           # max idxs per dma_gather instruction (SWDGE ring limit)
PAD_S = 255.0         # slot/sub id for padding positions


# ------------------------------------------------------------------ host prep
def _preprocess(x, edge_src, edge_dst, batch, num_graphs):
    N = x.shape[0]
    Npad = NCORES * NPC
    indeg = np.bincount(edge_dst, minlength=N).astype(np.int64)
    deg = indeg + 1  # self loop
    dinv = (1.0 / np.sqrt(deg.astype(np.float64))).astype(np.float32)

    # Two-tier greedy assignment of dst nodes to (core, tile) bins by
    # indegree: tiles 0..123 target just under 12 gather groups (1536 slots),
    # the last 4 tiles per core absorb the excess -> ~5% fewer pad slots.
    order = np.argsort(-indeg, kind="stable")
    nbins = NCORES * TT
    NHI = 4
    cap_lo = 12 * 128 - 10.0
    t_hi = (len(edge_dst) / NCORES - (TT - NHI) * cap_lo) / NHI
    target = np.full(TT, cap_lo)
    for b_hi in (0, 1, TT // 2, TT // 2 + 1):
        target[b_hi] = max(t_hi, cap_lo)
    heap = [(-target[b % TT], b) for b in range(nbins)]
    heapq.heapify(heap)
    counts = np.zeros(nbins, np.int64)
    loads = np.zeros(nbins, np.float64)
    gid = np.empty(N, np.int64)
    for v in order:
        key, b = heapq.heappop(heap)
        c, t = b // TT, b % TT
        gid[v] = c * NPC + counts[b] * TT + t
        counts[b] += 1
        loads[b] += indeg[v]
        if counts[b] < 128:
            heapq.heappush(heap, (loads[b] - target[t], b))
    assert counts.max() <= 128

    dinv_pad = np.ones(Npad, np.float32)
    dinv_pad[gid] = dinv

    sg, dg = gid[edge_src], gid[edge_dst]
    core_e = dg // NPC
    tile_e = dg % TT
    slot_e = (dg % NPC) // TT
    binid = core_e * TT + tile_e
    cnts = np.bincount(binid, minlength=nbins).reshape(NCORES, TT)
    Ls = (-(-cnts.max(axis=0) // 128) * 128).astype(np.int64)
    Ls[TT // 2 - 1] += (-Ls[:TT // 2].sum()) % 1024     # align half streams
    Ls[TT - 1] += (-Ls[TT // 2:].sum()) % 1024
    prefix = np.concatenate([[0], np.cumsum(Ls)])
    S = int(prefix[-1])

    ks = (16, 8, 4)
    order_e = np.argsort(binid, kind="stable")
    sortedbin = binid[order_e]
    run_start = np.searchsorted(sortedbin, np.arange(nbins))
    rank = np.arange(len(order_e)) - run_start[sortedbin]
    pos = prefix[sortedbin % TT] + rank
    core_arr = sortedbin // TT

    idx_flat = np.zeros((3, NCORES, S), np.int16)
    sub_flat = np.full((3, NCORES, S), PAD_S, np.float32)
    sval_flat = np.full((NCORES, S), PAD_S, np.float32)
    for c in range(NCORES):
        m = core_arr == c
        es, p = order_e[m], pos[m]
        sval_flat[c, p] = slot_e[es].astype(np.float32)
        for li, k in enumerate(ks):
            idx_flat[li, c, p] = (sg[es] // k).astype(np.int16)
            sub_flat[li, c, p] = (sg[es] % k).astype(np.float32)

    def wrap16(a):  # [S] -> [128, S//16]
        return np.tile(a.reshape(S // 16, 16).T, (8, 1)).copy()

    def wrap128(a):  # [S] -> [128, S//128]
        return a.reshape(S // 128, 128).T.astype(ml_dtypes.bfloat16)

    idx_tbl = np.stack([[wrap16(idx_flat[li, c]) for c in range(NCORES)]
                        for li in range(3)])
    sub_tbl = np.stack([[wrap128(sub_flat[li, c]) for c in range(NCORES)]
                        for li in range(3)])
    sval_tbl = np.stack([wrap128(sval_flat[c]) for c in range(NCORES)])

    x_perm = np.zeros((Npad, x.shape[1]), np.float32)
    x_perm[gid] = x
    dinv_all = dinv_pad.reshape(128, Npad // 128).astype(ml_dtypes.bfloat16)
    dinv_my = dinv_pad.reshape(NCORES, 128, TT).copy()
    dinv2_my = (dinv_my * dinv_my).copy()
    x_my = x_perm.reshape(NCORES, 128, TT * x.shape[1]).copy()    # [8,128,TT*8]

    cnt = np.bincount(batch, minlength=num_graphs).astype(np.float64)
    invc = (1.0 / np.maximum(cnt, 1.0)).astype(np.float64)
    bd = batch[edge_dst].astype(np.int64)
    w = dinv[edge_src].astype(np.float64) * dinv[edge_dst] * invc[bd]
    Pacc = np.bincount(sg * num_graphs + bd, weights=w,
                       minlength=Npad * num_graphs)
    Pacc += np.bincount(gid * num_graphs + batch.astype(np.int64),
                        weights=(dinv.astype(np.float64) ** 2) * invc[batch],
                        minlength=Npad * num_graphs)
    Pacc = Pacc.reshape(Npad, num_graphs).astype(np.float32)
    P_my = Pacc.reshape(NCORES, 128, TT * num_graphs).astype(ml_dtypes.bfloat16)

    x_perm = x_perm.astype(ml_dtypes.bfloat16)
    return dict(prefix=prefix, S=S, Npad=Npad, idx_tbl=idx_tbl, sub_tbl=sub_tbl,
                sval_tbl=sval_tbl, x_perm=x_perm, x_my=x_my,
                dinv_all=dinv_all, dinv_my=dinv_my, dinv2_my=dinv2_my,
                P_my=P_my)


# ------------------------------------------------------------------ device IR
def _build(meta, num_graphs, n_classes, alphas, n_cores=NCORES):
    S, Npad = meta["S"], meta["Npad"]
    prefix = [int(v) for v in meta["prefix"]]
    SH0 = prefix[TT // 2]
    Fs = [8, 16, 32]          # aggregation widths, layers 1-3
    Fos = [16, 32, 64]        # output widths, layers 1-3
    ks = [16, 8, 4]           # nodes per 256B gather row
    nodes_my = NPC

    nc = bacc.Bacc("TRN2", target_bir_lowering=False, debug=False,
                   num_devices=n_cores, num_swdge_queues=4)
    rg = [list(range(n_cores))]

    x_in = nc.dram_tensor("x_perm", [Npad, 8], BF16, kind="ExternalInput")
    xmy_in = nc.dram_tensor("x_my", [128, TT * 8], F32, kind="ExternalInput")
    dall_in = nc.dram_tensor("dinv_all", [128, Npad // 128], BF16,
                             kind="ExternalInput")
    dmy_in = nc.dram_tensor("dinv_my", [128, TT], F32, kind="ExternalInput")
    d2my_in = nc.dram_tensor("dinv2_my", [128, TT], F32, kind="ExternalInput")
    idx_in = [nc.dram_tensor(f"idx{l+1}", [128, S // 16], I16,
                             kind="ExternalInput") for l in range(3)]
    sub_in = [nc.dram_tensor(f"sub{l+1}", [128, S // 128], BF16,
                             kind="ExternalInput") for l in range(3)]
    sval_in = nc.dram_tensor("sval", [128, S // 128], BF16, kind="ExternalInput")
    P_in = nc.dram_tensor("P_my", [128, TT * num_graphs], BF16,
                          kind="ExternalInput")
    W_in = [nc.dram_tensor(f"W{l+1}", [Fs[l], Fos[l]], BF16,
                           kind="ExternalInput") for l in range(3)]
    b_in = [nc.dram_tensor(f"b{l+1}", [Fos[l], 1], F32, kind="ExternalInput")
            for l in range(3)]
    bn_in = [nc.dram_tensor(f"bn{l+1}", [Fos[l], 1], F32, kind="ExternalInput")
             for l in range(3)]
    W4_in = nc.dram_tensor("W4", [64, 128], F32, kind="ExternalInput")
    b4_in = nc.dram_tensor("b4", [128, 1], F32, kind="ExternalInput")
    Wlin_in = nc.dram_tensor("Wlin", [128, n_classes], F32, kind="ExternalInput")
    blin_in = nc.dram_tensor("blin_rep", [num_graphs, n_classes], F32,
                             kind="ExternalInput")
    out_t = nc.dram_tensor("out", [num_graphs, n_classes], F32,
                           kind="ExternalOutput")

    g1 = nc.dram_tensor("g1", [Npad, 8], BF16)
    g2 = nc.dram_tensor("g2", [Npad, 16], BF16, addr_space="Shared")
    g3 = nc.dram_tensor("g3", [Npad, 32], BF16, addr_space="Shared")
    gs = [g1, g2, g3]
    hs2 = nc.dram_tensor("hs2", [nodes_my, 16], BF16)
    hs3 = nc.dram_tensor("hs3", [nodes_my, 32], BF16)
    hss = [hs2, hs3]
    pooled_d = nc.dram_tensor("pooled", [128, num_graphs], F32)
    pooled_r = nc.dram_tensor("pooled_red", [128, num_graphs], F32, addr_space="Shared")

    with tile.TileContext(nc) as tc:
        with (
            tc.tile_pool(name="const", bufs=1) as cpool,
            tc.tile_pool(name="meta", bufs=2) as mpool,
            tc.tile_pool(name="gat", bufs=8) as gpool,
            tc.tile_pool(name="am", bufs=8) as apool,
            tc.tile_pool(name="red", bufs=4) as rpool,
            tc.tile_pool(name="big", bufs=1) as bpool,
            tc.tile_pool(name="ps", bufs=1, space="PSUM") as pspool,
        ):
            nc.gpsimd.load_library(mlp_lib)

            iden = cpool.tile([128, 128], BF16)
            make_identity(nc, iden[:])
            iota = cpool.tile([128, 128], BF16)
            nc.gpsimd.iota(iota[:], [[1, 128]], channel_multiplier=0,
                           allow_small_or_imprecise_dtypes=True)

            dinv_my = cpool.tile([128, TT], F32)
            nc.sync.dma_start(dinv_my[:], dmy_in.ap())
            dinv2_my = cpool.tile([128, TT], F32)
            nc.sync.dma_start(dinv2_my[:], d2my_in.ap())
            sval_sb = cpool.tile([128, S // 128], BF16)
            nc.scalar.dma_start(sval_sb[:], sval_in.ap())
            x_my = cpool.tile([128, TT * 8], F32)
            nc.scalar.dma_start(x_my[:], xmy_in.ap())
            P_sb = cpool.tile([128, TT * num_graphs], BF16)
            nc.scalar.dma_start(P_sb[:], P_in.ap())

            Wt, btl, bntl = [], [], []
            for l in range(3):
                w = cpool.tile([128, Fos[l]], BF16, tag=f"W{l}")
                nc.sync.dma_start(w[:Fs[l], :], W_in[l].ap())
                Wt.append(w)
                b = cpool.tile([128, 1], F32, tag=f"b{l}")
                nc.sync.dma_start(b[:Fos[l], :], b_in[l].ap())
                btl.append(b)
                bn = cpool.tile([128, 1], F32, tag=f"bn{l}")
                nc.sync.dma_start(bn[:Fos[l], :], bn_in[l].ap())
                bntl.append(bn)
            W4_sb = cpool.tile([128, 128], F32, tag="W4")
            nc.sync.dma_start(W4_sb[:64, :], W4_in.ap())
            b4_sb = cpool.tile([128, 1], F32, tag="b4")
            nc.sync.dma_start(b4_sb[:], b4_in.ap())
            Wlin_sb = cpool.tile([128, n_classes], F32, tag="wlin")
            nc.sync.dma_start(Wlin_sb[:], Wlin_in.ap())
            blin_sb = cpool.tile([num_graphs, n_classes], F32, tag="blin")
            nc.sync.dma_start(blin_sb[:], blin_in.ap())

            # ---------------- g1 = dinv * x (full, every core), bf16 packed
            ncols = Npad // 128
            CCH = 128
            xv = x_in.ap().rearrange("(p c) f -> p c f", p=128)
            g1v = g1.ap().rearrange("(p c) f -> p c f", p=128)
            for c0 in range(0, ncols, CCH):
                xt = mpool.tile([128, CCH, 8], BF16, tag="xt")
                nc.scalar.dma_start(xt[:], xv[:, c0:c0 + CCH, :])
                da = mpool.tile([128, CCH], BF16, tag="da")
                nc.sync.dma_start(da[:], dall_in.ap()[:, c0:c0 + CCH])
                gt = mpool.tile([128, CCH, 8], BF16, tag="gt")
                nc.vector.tensor_tensor(
                    gt[:], xt[:],
                    da[:, :, None].broadcast_to([128, CCH, 8]),
                    op=mybir.AluOpType.mult)
                nc.scalar.dma_start(g1v[:, c0:c0 + CCH, :], gt[:])

            gq = [0]
            aggT = bpool.tile([128, nodes_my // 2], BF16, tag="aggT")
            h_sbT = bpool.tile([128, nodes_my // 2], BF16, tag="h_sbT")
            agg = bpool.tile([128, TT * 32], BF16, tag="agg")
            gnext0 = bpool.tile([128, TT * 16], BF16, tag="gnext0")
            gnext1 = bpool.tile([128, TT * 32], BF16, tag="gnext1")
            h3buf = bpool.tile([128, TT * 64], BF16, tag="h3buf")
            IH = max(SH0, S - SH0) // 16
            idx_h = [bpool.tile([128, IH], I16, tag=f"idx_h{h}",
                                name=f"idx_h{h}") for h in range(2)]
            sub_d = [bpool.tile([128, S // 128], BF16, tag=f"sub_d{h}",
                                name=f"sub_d{h}") for h in range(2)]
            nc.sync.dma_start(idx_h[0][:, :SH0 // 16],
                              idx_in[0].ap()[:, :SH0 // 16])
            nc.sync.dma_start(idx_h[1][:, :(S - SH0) // 16],
                              idx_in[0].ap()[:, SH0 // 16:])
            nc.sync.dma_start(sub_d[0][:], sub_in[0].ap())
            pp_sb = cpool.tile([128, num_graphs], F32, tag="pp_sb")
            nc.vector.memset(pp_sb[:], 0.0)

            for li in range(3):
                F, Fo, k = Fs[li], Fos[li], ks[li]
                gprev = [None, gnext0, gnext1][li]
                gout = [gnext0, gnext1, None][li]
                sub_sb = sub_d[li % 2]
                gsrc = gs[li].ap().rearrange("(r k) f -> r (k f)", k=k)

                # ---- aggregation + post-phase, interleaved per node-half.
                # Flat 1024-idx gather stream; groups map to dst tiles via
                # position (tile t spans [t*L, (t+1)*L), L % 128 == 0).
                a_f = alphas[li]
                HT = TT // 2
                psums = {}

                def finish_group(g0, k=k, F=F, li=li, gprev=gprev):
                    # drain one 4-tile psum bank, slice-reduce each tile's k
                    # F-wide blocks (3D halving view), apply dinv scale and
                    # the self-loop term, write agg for 4 tiles at once.
                    t0 = 4 * g0
                    psum = psums.pop(g0)
                    red0 = rpool.tile([128, 512], F32, tag="red0")
                    nc.scalar.copy(red0[:], psum[:, :512])
                    cur = red0[:].rearrange("p (t m) -> p t m", t=4)
                    m = k * F
                    while m > F:
                        half = m // 2
                        nxt = rpool.tile([128, 4, 64], F32, tag=f"redt{m}")
                        nc.vector.tensor_tensor(
                            nxt[:, :, :half], cur[:, :, :half],
                            cur[:, :, half:m], op=mybir.AluOpType.add)
                        cur = nxt[:, :, :half]
                        m = half
                    sc4 = rpool.tile([128, 4, 32], F32, tag="sc4")
                    if li == 0:
                        nc.vector.tensor_tensor(
                            sc4[:, :, :F],
                            x_my[:, t0 * F:(t0 + 4) * F].rearrange(
                                "p (t f) -> p t f", f=F),
                            dinv2_my[:, t0:t0 + 4, None].broadcast_to(
                                [128, 4, F]),
                            op=mybir.AluOpType.mult)
                    else:
                        # gprev holds dinv*h, so one more dinv = dinv^2*h
                        nc.vector.tensor_tensor(
                            sc4[:, :, :F],
                            gprev[:, t0 * F:(t0 + 4) * F].rearrange(
                                "p (t f) -> p t f", f=F),
                            dinv_my[:, t0:t0 + 4, None].broadcast_to(
                                [128, 4, F]),
                            op=mybir.AluOpType.mult)
                    t24 = rpool.tile([128, 4, 32], F32, tag="t24")
                    nc.vector.tensor_tensor(
                        t24[:, :, :F], cur[:, :, :F],
                        dinv_my[:, t0:t0 + 4, None].broadcast_to([128, 4, F]),
                        op=mybir.AluOpType.mult)
                    nc.vector.tensor_tensor(
                        agg[:, t0 * F:(t0 + 4) * F].rearrange(
                            "p (t f) -> p t f", f=F),
                        t24[:, :, :F], sc4[:, :, :F],
                        op=mybir.AluOpType.add)

                cur_t = [0]
                for hb in range(2):
                  if True:
                    idx_sb = idx_h[hb]
                    hb0 = 0 if hb == 0 else SH0
                    hb1 = SH0 if hb == 0 else S
                    for base in range(hb0, hb1, GIDX):
                        nidx = GIDX
                        ng = nidx // 128
                        gt = gpool.tile([128, 8, 128], BF16, tag="gtile")
                        nc.gpsimd.dma_gather(
                            gt[:, :ng, :], gsrc,
                            idx_sb[:, (base - hb0) // 16:
                                   (base - hb0 + nidx) // 16],
                            nidx, nidx, 128, queue_num=gq[0] % 4)
                        gq[0] += 1
                        sv = sval_sb[:, base // 128:(base + nidx) // 128]
                        sb = sub_sb[:, base // 128:(base + nidx) // 128]
                        A = apool.tile([128, 8, 128], FP8, tag="A")
                        nc.vector.tensor_tensor(
                            A[:, :ng, :],
                            sv[:, :, None].broadcast_to([128, ng, 128]),
                            iota[:, None, :].broadcast_to([128, ng, 128]),
                            op=mybir.AluOpType.is_equal)
                        Mt = apool.tile([128, 128], BF16, tag="M")
                        nc.vector.tensor_tensor(
                            Mt[:, :ng * k].rearrange("p (g k) -> p g k", k=k),
                            sb[:, :, None].broadcast_to([128, ng, k]),
                            iota[:, None, :k].broadcast_to([128, ng, k]),
                            op=mybir.AluOpType.is_equal)
                        Gm = gpool.tile([128, 8, 128], FP8, tag="Gm")
                        nc.vector.tensor_tensor(
                            Gm[:, :ng, :].rearrange("p g (k f) -> p (g k) f",
                                                    k=k),
                            gt[:, :ng, :].rearrange("p g (k f) -> p (g k) f",
                                                    k=k),
                            Mt[:, :ng * k, None].broadcast_to([128, ng * k, F]),
                            op=mybir.AluOpType.mult)
                        for gg in range(ng):
                            pos = base + gg * 128
                            while pos >= prefix[cur_t[0] + 1]:
                                cur_t[0] += 1
                            t = cur_t[0]
                            g4 = t // 4
                            if g4 not in psums:
                                psums[g4] = pspool.tile(
                                    [128, 512], F32, tag=f"ps{g4 % 6}",
                                    name=f"pst{g4 % 6}")
                            c4 = (t % 4) * 128
                            nc.tensor.matmul(
                                psums[g4][:, c4:c4 + 128],
                                A[:, gg, :], Gm[:, gg, :],
                                start=(pos == prefix[t]),
                                stop=(pos == prefix[t + 1] - 128),
                                skip_group_check=True)
                            if pos == prefix[4 * g4 + 4] - 128:
                                finish_group(g4)
                    if li < 2:
                        nc.sync.dma_start(
                            idx_h[hb][:, :(hb1 - hb0) // 16],
                            idx_in[li + 1].ap()[:, hb0 // 16:hb1 // 16])
                        if hb == 0:
                            nc.sync.dma_start(sub_d[(li + 1) % 2][:],
                                              sub_in[li + 1].ap())

                  # ---- this half: transpose agg -> aggT, W matmul+PReLU,
                  #      transpose back, keep h_res, emit gnext / P-psum
                  if True:
                    t0 = hb * HT
                    for tt in range(HT):
                        t = t0 + tt
                        tp = pspool.tile([128, 512], BF16, tag=f"ps{6 + tt % 2}")
                        nc.tensor.matmul(tp[:F, :128],
                                         agg[:, t * F:(t + 1) * F],
                                         iden[:], is_transpose=True,
                                         skip_group_check=True)
                        nc.scalar.copy(aggT[:F, tt * 128:(tt + 1) * 128],
                                       tp[:F, :128])
                    for n0 in range(0, nodes_my // 2, 512):
                        hp = pspool.tile([128, 512], F32,
                                         tag=f"ps{6 + (n0 // 512) % 2}")
                        nc.tensor.matmul(hp[:Fo, :512], Wt[li][:F, :Fo],
                                         aggT[:F, n0:n0 + 512],
                                         skip_group_check=True)
                        # prelu(x+b) = relu(x+b) - a * relu(-x-b)
                        nc.scalar.activation(
                            h_sbT[:Fo, n0:n0 + 512], hp[:Fo, :512],
                            mybir.ActivationFunctionType.Relu,
                            bias=btl[li][:Fo, :], scale=1.0)
                        hrelu = mpool.tile([128, 512], BF16, tag="hrelu")
                        nc.scalar.activation(
                            hrelu[:Fo, :512], hp[:Fo, :512],
                            mybir.ActivationFunctionType.Relu,
                            bias=bntl[li][:Fo, :], scale=-1.0)
                        nc.vector.scalar_tensor_tensor(
                            h_sbT[:Fo, n0:n0 + 512], hrelu[:Fo, :512],
                            float(-a_f), h_sbT[:Fo, n0:n0 + 512],
                            op0=mybir.AluOpType.mult, op1=mybir.AluOpType.add)
                    for tt in range(HT):
                        t = t0 + tt
                        tb = pspool.tile([128, 512], BF16, tag=f"ps{6 + tt % 2}")
                        nc.tensor.matmul(tb[:128, :Fo],
                                         h_sbT[:Fo, tt * 128:(tt + 1) * 128],
                                         iden[:Fo, :Fo], is_transpose=True,
                                         skip_group_check=True)
                        if li < 2:
                            nc.scalar.activation(
                                gout[:, t * Fo:(t + 1) * Fo], tb[:, :Fo],
                                mybir.ActivationFunctionType.Identity,
                                scale=dinv_my[:, t:t + 1])
                        else:
                            nc.vector.tensor_copy(
                                h3buf[:, t * Fo:(t + 1) * Fo], tb[:, :Fo])
                            ppp = pspool.tile([128, 512], F32,
                                              tag=f"ps{6 + (tt + 1) % 2}")
                            nc.tensor.matmul(
                                ppp[:64, :num_graphs],
                                h3buf[:, t * Fo:(t + 1) * Fo],
                                P_sb[:, t * num_graphs:(t + 1) * num_graphs],
                                start=True, stop=True,
                                skip_group_check=True)
                            nc.vector.tensor_tensor(
                                pp_sb[:64, :], pp_sb[:64, :],
                                ppp[:64, :num_graphs],
                                op=mybir.AluOpType.add)

                    if li < 2:
                        Foh = Fos[li]
                        th0 = hb * HT
                        hsv = hss[li].ap().rearrange("(p t) f -> p t f", p=128)
                        nc.sync.dma_start(
                            hsv[:, th0:th0 + HT, :],
                            gout[:, th0 * Foh:(th0 + HT) * Foh].rearrange(
                                "p (t f) -> p t f", f=Foh))
                if li < 2:
                    if n_cores > 1:
                        nc.gpsimd.collective_compute(
                            "AllGather", mybir.AluOpType.bypass, rg,
                            [hss[li].ap()], [gs[li + 1].ap()])
                    else:
                        nc.sync.dma_start(gs[li + 1].ap()[:nodes_my, :],
                                          hss[li].ap())

            # ---------------- tail: pooledT = W4^T @ ppT + b4; AllReduce; lin
            poolp = pspool.tile([128, 512], F32, tag="ps7")
            nc.tensor.matmul(poolp[:128, :num_graphs], W4_sb[:64, :128],
                             pp_sb[:64, :num_graphs], skip_group_check=True)
            poolT_sb = cpool.tile([128, num_graphs], F32, tag="poolT")
            nc.scalar.activation(poolT_sb[:], poolp[:128, :num_graphs],
                                 mybir.ActivationFunctionType.Identity,
                                 bias=b4_sb[:, :], scale=1.0)
            if n_cores > 1:
                nc.sync.dma_start(pooled_d.ap(), poolT_sb[:])
                nc.gpsimd.collective_compute(
                    "AllReduce", mybir.AluOpType.add, rg,
                    [pooled_d.ap()], [pooled_r.ap()])
                poolT2 = cpool.tile([128, num_graphs], F32, tag="poolT2")
                nc.sync.dma_start(poolT2[:], pooled_r.ap())
            else:
                poolT2 = poolT_sb
            fin = pspool.tile([128, 512], F32, tag="ps6")
            nc.tensor.matmul(fin[:num_graphs, :n_classes], poolT2[:],
                             Wlin_sb[:], skip_group_check=True)
            out_sb = cpool.tile([num_graphs, n_classes], F32, tag="outsb")
            nc.vector.tensor_tensor(out_sb[:], fin[:num_graphs, :n_classes],
                                    blin_sb[:], op=mybir.AluOpType.add)
            nc.sync.dma_start(out_t.ap(), out_sb[:])

    nc.compile()
    return nc


# ------------------------------------------------------------------ entry
def kernel(x, edge_src, edge_dst, batch,
           W1, b1, W2, b2, W3, b3, W4, b4,
           a1, a2, a3, Wlin, blin, n_cores=NCORES):
    x = np.asarray(x, dtype=np.float32)
    edge_src = np.asarray(edge_src, dtype=np.int32)
    edge_dst = np.asarray(edge_dst, dtype=np.int32)
    batch = np.asarray(batch, dtype=np.int32)
    Ws = [np.asarray(w, np.float32) for w in (W1, W2, W3, W4)]
    bs = [np.asarray(b, np.float32) for b in (b1, b2, b3, b4)]
    alphas = [float(a1), float(a2), float(a3)]
    Wlin = np.asarray(Wlin, np.float32)
    blin = np.asarray(blin, np.float32)
    NG, NCLS = 64, Wlin.shape[1]

    meta = _preprocess(x, edge_src, edge_dst, batch, NG)
    nc = _build(meta, NG, NCLS, alphas, n_cores)
    in_maps = _in_maps(meta, Ws, bs, Wlin, blin, NG, n_cores)
    res = run_bass_kernel_spmd(nc, in_maps, core_ids=list(range(n_cores)))
    return np.asarray(res.results[0]["out"], dtype=np.float32)


def _in_maps(meta, Ws, bs, Wlin, blin, NG, n_cores=NCORES):
    in_maps = []
    for c in range(n_cores):
        m = dict(
            x_perm=meta["x_perm"],
            x_my=np.ascontiguousarray(meta["x_my"][c]),
            dinv_all=meta["dinv_all"],
            dinv_my=np.ascontiguousarray(meta["dinv_my"][c]),
            dinv2_my=np.ascontiguousarray(meta["dinv2_my"][c]),
            sval=np.asarray(meta["sval_tbl"][c]),
            P_my=np.asarray(meta["P_my"][c]),
            W4=Ws[3].astype(np.float32),
            b4=np.ascontiguousarray(bs[3].reshape(-1, 1)),
            Wlin=Wlin,
            blin_rep=np.tile(blin[None, :], (NG, 1)).astype(np.float32),
        )
        for l in range(3):
            m[f"idx{l+1}"] = np.asarray(meta["idx_tbl"][l, c])
            m[f"sub{l+1}"] = np.asarray(meta["sub_tbl"][l][c])
            m[f"W{l+1}"] = Ws[l].astype(ml_dtypes.bfloat16)
            m[f"b{l+1}"] = np.ascontiguousarray(bs[l].reshape(-1, 1))
            m[f"bn{l+1}"] = np.ascontiguousarray(-bs[l].reshape(-1, 1))
        in_maps.append(m)
    return in_maps
